# revision 91
# baseline (speedup 1.0000x reference)
"""EntityAwareAttention Trainium2 kernel, v3.

Per batch b of B=2048:
    hid_e{1,2} = hidden[b, e{1,2}_idx[b]]                       # [600]
    e{1,2}_type = softmax(hid_e @ LT.T) @ LT                    # [600], T=3
    u1 = concat(hidden, pos1, pos2) @ W_hid.T                   # [128, 50]
    u2 = concat(hid_e1, e1_type, hid_e2, e2_type) @ W_ent.T     # [50]
    u = tanh(u1 + u2); scores = u @ v; alpha = softmax(scores)  # [128]
    z = alpha @ hidden[b]                                       # [600]

Pure data parallel over batch: 8 cores x 256 batches, weights replicated.

v3 design (~2x the v2 kernel on the CoreSim cost model):
  - v2's bottleneck was PSUM evacuation of on-chip PE transposes
    (DVE 87% / Act 85% busy, nearly all tensor-copy).  v3 deletes the
    transpose pipeline entirely: the host pre-packs hidden a second
    time in feature-major fp8 (ht8 [128, 6ch, BC*L], pos folded in as
    chunk 5), DMA'd straight into the u1 rhs slot.  Token-major bf16
    hidden is still loaded for the z path (z matmuls have free-size-1
    outputs, which the PE does at negligible cost).
  - u1 matmuls in fp8 DoubleRow, group pairs stacked vertically in one
    PSUM bank (rows 0:64 / 64:128) so a single tanh covers 8 batches;
    v is host-replicated to partitions 64-113 so the per-batch score
    matmuls can read either half.
  - Scores lag one group pair (carried across rounds) so the PE never
    waits on tanh; softmax is unnormalized (host divides); z goes
    PSUM -> DRAM directly, batched 2 rounds per DMA; esum accumulates
    on-chip all 8 rounds and ships once.
  - DMA is the cost floor (hidden 1.5 copies + pos ~ 24.3us/round of
    queue time) and only SP/Act/Pool can issue DMAs, so loads are
    split SP: 3 hT chunks + 9 hp batches, Act: 1 + 11 (Act also runs
    tanh/exp), Pool: 2 + 12 (+ gathers and stores).  Entity/u2 chain
    unchanged from v2 except PSUM evacs moved Act -> DVE (DVE is
    otherwise idle; Act is a DMA queue now).
"""

import numpy as np

B, L, H2, PP, A, T = 2048, 128, 600, 50, 50, 3
NCORES = 8
BC = B // NCORES   # 256 batches per core
SB = 128           # superbatch for the entity/u2 pipeline
ROUND = 32         # batches per round
GROUP = 4          # batches per u1 matmul group (N = 4*128 = 512)
NPAIR = ROUND // (2 * GROUP)  # group pairs per round
NR = BC // ROUND   # rounds per core
NCH = 6            # rhs feature chunks (5 hidden + 1 pos)
HCH = 5            # hidden chunks (4x128 + 88)
EPAD = 640         # entity vectors padded to 5x128
ECH = 5
POSF = 2 * PP      # 100 pos features

# DMA queue split for the per-round loads (SP / Act / Pool)
HT_SPLIT = (3, 1, 2)    # of the 6 ht8 chunks
HP_SPLIT = (10, 10, 12)  # of the 32 hp batches
LAST_HALVES = True      # finish the last round in halves (shorter tail)

_CACHE = {}


def _build_bass():
    import concourse.bass as bass
    import concourse.bacc as bacc
    import concourse.tile as tile
    from concourse import mybir
    from concourse.masks import make_identity

    f32 = mybir.dt.float32
    bf16 = mybir.dt.bfloat16
    fp8 = mybir.dt.float8e4
    i32 = mybir.dt.int32
    AF = mybir.ActivationFunctionType
    AX = mybir.AxisListType
    DR = mybir.MatmulPerfMode.DoubleRow

    nc = bacc.Bacc("TRN2", debug=False, target_bir_lowering=False)

    # hidden is staged l-major ([L, BC, H2]) and ht8 round-major so each
    # round's load is one long contiguous run per partition
    hid_d = nc.dram_tensor("hid_l", [L, BC, H2], bf16, kind="ExternalInput").ap()
    ht8_d = nc.dram_tensor(
        "ht8r", [NR, 128, NCH * ROUND * L], fp8, kind="ExternalInput").ap()
    e1r_d = nc.dram_tensor("e1rows", [BC, 1], i32, kind="ExternalInput").ap()
    e2r_d = nc.dram_tensor("e2rows", [BC, 1], i32, kind="ExternalInput").ap()
    # host-pretransposed weights
    whidT_d = nc.dram_tensor("whidT", [128, NCH, 64], fp8, kind="ExternalInput").ap()
    wentT_d = nc.dram_tensor("wentT", [128, 4 * ECH, A], bf16, kind="ExternalInput").ap()
    ltT_d = nc.dram_tensor("ltT", [128, ECH, T], bf16, kind="ExternalInput").ap()
    lt16_d = nc.dram_tensor("lt16", [T, H2], bf16, kind="ExternalInput").ap()
    v_d = nc.dram_tensor("v128", [128, 1], bf16, kind="ExternalInput").ap()
    z_d = nc.dram_tensor(
        "z", [NR // 2, 128, ECH, 2, ROUND], f32, kind="ExternalOutput"
    ).ap()
    # per-batch softmax denominators; z is stored unnormalized and the
    # host divides (keeps the recip/scale off the round critical path)
    # rows 32:48 alias the last round's second half (a partition-start-16
    # write is not ISA-legal, so it lands at partition 32 instead)
    es_d = nc.dram_tensor(
        "esum", [ROUND + 16, NR], f32, kind="ExternalOutput").ap()

    hid_flat = hid_d.rearrange("l b d -> (l b) d")

    with tile.TileContext(nc) as tc, nc.allow_non_contiguous_dma(
        "partition-dim-last APs for the parallel DMA fabric"
    ):
        with (
            tc.tile_pool(name="const", bufs=1) as const,
            tc.tile_pool(name="hp_pool", bufs=3) as hp_pool,
            tc.tile_pool(name="ht_pool", bufs=3) as ht_pool,
            tc.tile_pool(name="u_pool", bufs=2) as u_pool,
            tc.tile_pool(name="ent_pool", bufs=2) as ent_pool,
            tc.tile_pool(name="small", bufs=4) as small,
            tc.tile_pool(name="zs_pool", bufs=2) as zs_pool,
            tc.tile_pool(name="ps_u1", bufs=2, space="PSUM") as ps_u1,
            tc.tile_pool(name="ps_h", bufs=2, space="PSUM") as ps_h,
            tc.tile_pool(name="ps_sc", bufs=2, space="PSUM") as ps_sc,
            tc.tile_pool(name="ps_z", bufs=1, space="PSUM") as ps_z,
            tc.tile_pool(name="ps_misc", bufs=1, space="PSUM") as ps_misc,
        ):
            # ---------------- constants (all host-prepacked) ----------------
            id_f32 = const.tile([128, 128], f32)
            make_identity(nc, id_f32[:, :])
            id_bf = const.tile([128, 128], bf16)
            nc.vector.tensor_copy(id_bf[:, :], id_f32[:, :])

            # const loads spread across the three DMA queues so no single
            # queue delays the round-0 loads by the full preamble
            whidT = const.tile([128, NCH, 64], fp8)
            nc.sync.dma_start(out=whidT[:, :, :], in_=whidT_d)
            wentT = const.tile([128, 4 * ECH, A], bf16)
            nc.scalar.dma_start(out=wentT[:, :, :], in_=wentT_d)
            v128 = const.tile([128, 1], bf16)
            nc.scalar.dma_start(out=v128[:, :], in_=v_d)
            ltT = const.tile([128, ECH, T], bf16)
            nc.sync.dma_start(out=ltT[:, :, :], in_=ltT_d)
            lt16 = const.tile([T, H2], bf16)
            nc.sync.dma_start(out=lt16[:, :], in_=lt16_d)
            # preload the activation table while DMAs warm up (one table
            # serves Tanh and Exp; otherwise the 1.3us load lands in the
            # middle of the entity chain)
            atl = const.tile([1, 1], f32)
            nc.scalar.activation(atl[:, :], id_f32[0:1, 0:1], AF.Tanh)
            esall = const.tile([ROUND + 16, NR], f32)
            nc.gpsimd.memset(esall[:, :], 0.0)  # rows 16:32 of the last
            # column are dead (their halves land in alias rows 32:48)

            def entity_block(s, out):
                """Gather + latent-type + u2 for superbatch s (128 batches).
                Generator: yields between cross-engine stages so the driver
                can interleave them with round groups (keeps the serial
                chain out of PE's in-order queue).  Stores the u2 tile in
                out["u2sb"]."""
                srcT = []
                tiles = []
                for rows_d in (e1r_d, e2r_d):
                    rows = ent_pool.tile([SB, 1], i32, tag="rows")
                    nc.sync.dma_start(
                        out=rows[:, :], in_=rows_d[s * SB:(s + 1) * SB, :]
                    )
                    ent = ent_pool.tile([SB, H2], bf16, tag="ent")
                    nc.gpsimd.indirect_dma_start(
                        out=ent[:, 0:H2],
                        out_offset=None,
                        in_=hid_flat,
                        in_offset=bass.IndirectOffsetOnAxis(ap=rows[:, 0:1], axis=0),
                    )
                    tiles.append(ent)
                yield
                # chunk 4 is 88 wide (600 = 4*128 + 88); no pad, no memset
                CW = [min(128, H2 - c * 128) for c in range(ECH)]
                for ent in tiles:
                    entT = ent_pool.tile([128, ECH, SB], bf16, tag="entT")
                    tp = ps_misc.tile([128, ECH, SB], bf16, tag="misc")
                    for c in range(ECH):
                        nc.tensor.transpose(
                            tp[0:CW[c], c, :],
                            ent[:, c * 128:c * 128 + CW[c]], id_bf[:, :]
                        )
                    nc.vector.tensor_copy(entT[:, 0:4, :], tp[:, 0:4, :])
                    nc.vector.tensor_copy(entT[0:CW[4], 4, :], tp[0:CW[4], 4, :])
                    yield
                    # latent-type logits [3, 128]
                    lg_ps = ps_misc.tile([T, SB], f32, tag="misc")
                    for c in range(ECH):
                        nc.tensor.matmul(
                            lg_ps[:, :], lhsT=ltT[0:CW[c], c, :],
                            rhs=entT[0:CW[c], c, :],
                            start=(c == 0), stop=(c == ECH - 1),
                        )
                    lgT_sb = ent_pool.tile([T, SB], f32, tag="lgT")
                    nc.vector.tensor_copy(lgT_sb[:, :], lg_ps[:, :])
                    yield
                    lg2_ps = ps_misc.tile([SB, T], f32, tag="misc")
                    nc.tensor.transpose(lg2_ps[:, :], lgT_sb[:, :], id_f32[0:T, 0:T])
                    expl = ent_pool.tile([SB, T], f32, tag="expl")
                    nc.scalar.activation(expl[:, :], lg2_ps[:, :], AF.Exp)
                    yield
                    ssum = ent_pool.tile([SB, 1], f32, tag="ssum")
                    nc.vector.reduce_sum(ssum[:, :], expl[:, :], axis=AX.X)
                    srec = ent_pool.tile([SB, 1], f32, tag="srec")
                    nc.vector.reciprocal(srec[:, :], ssum[:, :])
                    attw = ent_pool.tile([SB, T], f32, tag="attw")
                    nc.vector.tensor_scalar_mul(attw[:, :], expl[:, :], srec[:, 0:1])
                    yield
                    awT_ps = ps_misc.tile([T, SB], f32, tag="misc")
                    nc.tensor.transpose(awT_ps[:, :], attw[:, :], id_f32[:, :])
                    awT = ent_pool.tile([T, SB], bf16, tag="awT_sb")
                    nc.vector.tensor_copy(awT[:, :], awT_ps[:, :])
                    yield
                    # e_type = attw @ LT : [128, 600]
                    et = ent_pool.tile([SB, H2], bf16, tag="et_sb")
                    et_lo = ps_misc.tile([SB, 512], f32, tag="misc")
                    nc.tensor.matmul(
                        et_lo[:, :], lhsT=awT[:, :], rhs=lt16[:, 0:512],
                        start=True, stop=True,
                    )
                    nc.vector.tensor_copy(et[:, 0:512], et_lo[:, :])
                    yield
                    et_hi = ps_misc.tile([SB, 128], f32, tag="misc")
                    nc.tensor.matmul(
                        et_hi[:, 0:H2 - 512], lhsT=awT[:, :], rhs=lt16[:, 512:H2],
                        start=True, stop=True,
                    )
                    nc.vector.tensor_copy(et[:, 512:H2], et_hi[:, 0:H2 - 512])
                    yield
                    etT = ent_pool.tile([128, ECH, SB], bf16, tag="etT")
                    tp2 = ps_misc.tile([128, ECH, SB], bf16, tag="misc")
                    for c in range(ECH):
                        nc.tensor.transpose(
                            tp2[0:CW[c], c, :],
                            et[:, c * 128:c * 128 + CW[c]], id_bf[:, :]
                        )
                    nc.vector.tensor_copy(etT[:, 0:4, :], tp2[:, 0:4, :])
                    nc.vector.tensor_copy(etT[0:CW[4], 4, :], tp2[0:CW[4], 4, :])
                    yield
                    srcT.append((entT, etT))

                u2_ps = ps_misc.tile([A, SB], f32, tag="misc")
                order = [srcT[0][0], srcT[0][1], srcT[1][0], srcT[1][1]]
                k = 0
                for q in range(4):
                    for c in range(ECH):
                        cw = min(128, H2 - c * 128)
                        nc.tensor.matmul(
                            u2_ps[:, :],
                            lhsT=wentT[0:cw, q * ECH + c, :],
                            rhs=order[q][0:cw, c, :],
                            start=(k == 0), stop=(k == 19),
                        )
                        k += 1
                # padded to 64 rows (rows 50:64 zero) so the fused
                # relocate+u2-add and the id-matmul can cover the DR pad
                u2sb = ent_pool.tile([64, SB], bf16, tag="u2sb")
                nc.gpsimd.memset(u2sb[32:64, :], 0.0)
                nc.vector.tensor_copy(u2sb[0:A, :], u2_ps[:, :])
                out["u2sb"] = u2sb

            def load_hp(eng, ridx, hp, a, b):
                b0 = ridx * ROUND
                eng.dma_start(
                    out=hp[:, a:b, :],
                    in_=hid_d[:, b0 + a:b0 + b, :],
                )

            def emit_scores(sc_ps, pr, uT):
                for j in range(2 * GROUP):
                    half, jj = divmod(j, GROUP)
                    off = 64 * half
                    bl = pr * 2 * GROUP + j
                    nc.tensor.matmul(
                        sc_ps[:, bl:bl + 1],
                        lhsT=uT[off:off + A, jj * L:(jj + 1) * L],
                        rhs=v128[off:off + A, 0:1],
                        start=True, stop=True,
                    )

            carry = [None]  # (sc_ps, pair, uT) with scores not yet emitted

            def emit_groups(ridx, hp, hT, u2sb_fn, drain=None, nxt=None,
                            fin=None, last=False):
                """u1 + tanh for round ridx; group pairs share one PSUM bank
                (rows 0:64 / 64:128) so one tanh covers 8 batches.  Scores
                lag one pair, carried across rounds."""
                s, r = divmod(ridx, SB // ROUND)
                sc_ps = ps_sc.tile([L, ROUND], f32, tag="scT")
                for pr in range(NPAIR):
                    # group pair stacked on partitions (rows 0:64 / 64:128).
                    # The ISA requires matmul dst partition 0, so the odd
                    # group lands in a scratch bank and the otherwise-idle
                    # DVE relocates it — fusing in that group's u2 add for
                    # free; one tanh then covers 8 batches.
                    u1_ps = ps_u1.tile([128, GROUP * L], f32, tag="u1like")
                    hb_ps = ps_h.tile([64, GROUP * L], f32, tag="u1hi")
                    u2sb16 = u2sb_fn()
                    for half in range(2):
                        g = 2 * pr + half
                        dst = u1_ps if half == 0 else hb_ps
                        gsl = slice(g * GROUP * L, (g + 1) * GROUP * L)
                        for c in range(3):
                            nc.tensor.matmul(
                                dst[0:64, :],
                                lhsT=whidT[:, 2 * c:2 * c + 2, :],
                                rhs=hT[:, 2 * c:2 * c + 2, gsl],
                                start=(c == 0),
                                stop=(c == 2 and half == 1),
                                perf_mode=DR, skip_group_check=True,
                            )
                        b0r = r * ROUND + g * GROUP
                        u2r = u2sb16[0:64, b0r:b0r + GROUP]
                        u2b = bass.AP(
                            tensor=u2r.tensor, offset=u2r.offset,
                            ap=[u2r.ap[0], u2r.ap[1], [0, L]],
                        )
                        if half == 0:
                            # += u2 broadcast over tokens via identity matmul
                            nc.tensor.matmul(
                                u1_ps[0:64, :], lhsT=id_bf[0:64, 0:64],
                                rhs=u2b, start=False, stop=True,
                                skip_group_check=True,
                            )
                        else:
                            # relocate + u2 add in one DVE pass
                            nc.vector.scalar_tensor_tensor(
                                u1_ps[64:128, :].rearrange(
                                    "p (i l) -> p i l", i=GROUP),
                                hb_ps[:, :].rearrange(
                                    "p (i l) -> p i l", i=GROUP),
                                0.0, u2b,
                                op0=mybir.AluOpType.bypass,
                                op1=mybir.AluOpType.add,
                            )
                    uT = u_pool.tile([128, GROUP * L], bf16, tag="uT")
                    nc.scalar.activation(uT[:, :], u1_ps[:, :], AF.Tanh)
                    if drain is not None:
                        next(drain, None)
                        next(drain, None)
                    if nxt is not None:
                        # next rounds' Act/Pool loads, sliced between tanhs
                        # (hT prefetches two rounds ahead, hp one)
                        hp1, hT2, r1, r2 = nxt
                        p0, p1 = HP_SPLIT[0], HP_SPLIT[0] + HP_SPLIT[1]
                        mid = (p0 + p1) // 2
                        pm = (p1 + ROUND) // 2
                        if pr == 0:
                            if hT2 is not None:
                                load_act_ht(r2, hT2)
                        elif pr == 1:
                            if hp1 is not None:
                                load_hp(nc.scalar, r1, hp1, p0, mid)
                                load_hp(nc.gpsimd, r1, hp1, p1, pm)
                        elif pr == 2:
                            if hp1 is not None:
                                load_hp(nc.scalar, r1, hp1, mid, p1)
                                load_hp(nc.gpsimd, r1, hp1, pm, ROUND)
                    if carry[0] is not None:
                        emit_scores(*carry[0])
                        carry[0] = None
                    if last:
                        # no score lag in the last round: finish in halves
                        # as the scores become available to shorten the tail
                        emit_scores(sc_ps, pr, uT)
                        if pr == 1:
                            if fin is not None:
                                fin()
                            finish_round(ridx, hp, sc_ps, 0, ROUND // 2)
                        elif pr == NPAIR - 1:
                            finish_round(ridx, hp, sc_ps, ROUND // 2, ROUND)
                    else:
                        carry[0] = (sc_ps, pr, uT)
                        if pr == 1 and fin is not None:
                            fin()
                return hp, sc_ps

            zcur = [None, None]  # [zt_sb pair tile, zt_ps round tile]

            def finish_round(ridx, hp, sc_ps, q0=0, q1=ROUND):
                """Softmax numerator + z for batches q0:q1 of one round.
                Normally emitted (whole round) inside the NEXT round's
                group stream; the last round is finished in halves to
                shorten the serial tail.  z accumulates in PSUM across a
                round pair and ships one DMA per pair."""
                zslot = ridx % 2
                if q0 == 0:
                    if zslot == 0:
                        zsb_new = zs_pool.tile(
                            [128, ECH, 2, ROUND], f32, tag="zt_sb")
                        zcur[0] = zsb_new
                    zps_new = ps_z.tile([128, ECH, ROUND], f32, tag="zt")
                    zcur[1] = zps_new
                zt_sb, zt_ps = zcur
                n = q1 - q0
                scT_sb = small.tile([L, n], bf16, tag="scT_sb")
                nc.vector.tensor_copy(scT_sb[:, :], sc_ps[:, q0:q1])
                sc2_ps = ps_misc.tile([n, L], bf16, tag="misc")
                nc.tensor.transpose(sc2_ps[:, :], scT_sb[:, :], id_bf[:, :])
                exps = small.tile([n, L], bf16, tag="exps")
                nc.scalar.activation(exps[:, :], sc2_ps[:, :], AF.Exp)
                er0 = q0 if q0 % 32 == 0 else 32
                nc.vector.reduce_sum(esall[er0:er0 + n, ridx:ridx + 1],
                                     exps[:, :], axis=AX.X)
                aT_ps = ps_misc.tile([L, n], bf16, tag="misc")
                nc.tensor.transpose(aT_ps[:, :], exps[:, :], id_bf[0:n, 0:n])
                alphaT = small.tile([L, n], bf16, tag="alphaT")
                nc.vector.tensor_copy(alphaT[:, :], aT_ps[:, :])

                # zT[d, b] = sum_l hp[l, b, d] * exps[l, b]  (unnormalized)
                # chunk 4 covers features 472:600 (overlapping chunk 3) so
                # every PSUM row is written; the host drops the overlap
                for q in range(q0, q1):
                    for c in range(HCH):
                        oc = c * 128 if c < 4 else H2 - 128
                        nc.tensor.matmul(
                            zt_ps[:, c, q:q + 1],
                            lhsT=hp[:, q, oc:oc + 128],
                            rhs=alphaT[:, q - q0:q - q0 + 1],
                            start=True, stop=True,
                        )
                nc.vector.tensor_copy(
                    zt_sb[:, :, zslot, q0:q1], zt_ps[:, :, q0:q1])
                if zslot == 1 and q1 == ROUND:
                    nc.gpsimd.dma_start(
                        out=z_d[ridx // 2], in_=zt_sb[:, :, :, :]
                    )

            def ht_dma(eng, ridx, hT, ca, cb):
                eng.dma_start(
                    out=hT[:, ca:cb, :],
                    in_=ht8_d[ridx][:, ca * ROUND * L:cb * ROUND * L].rearrange(
                        "p (c n) -> p c n", c=cb - ca),
                )

            def load_ht_head(ridx, hT):
                """SP + Pool shares of round ridx's hT chunks."""
                c0, c1 = HT_SPLIT[0], HT_SPLIT[0] + HT_SPLIT[1]
                ht_dma(nc.sync, ridx, hT, 0, c0)
                ht_dma(nc.gpsimd, ridx, hT, c1, NCH)

            def load_act_ht(ridx, hT):
                c0, c1 = HT_SPLIT[0], HT_SPLIT[0] + HT_SPLIT[1]
                ht_dma(nc.scalar, ridx, hT, c0, c1)

            # ---------------- main schedule ----------------
            ent0, ent1 = {}, {}
            gen0 = entity_block(0, ent0)
            next(gen0)  # issue the gathers before anything else
            # hT is prefetched two rounds deep (it gates each round's start);
            # hp only one (first read a round and a half later, by z)
            hts = {}
            hps = {}
            for r0 in range(2):
                ht_t = ht_pool.tile([128, NCH, ROUND * L], fp8, tag="hT")
                hts[r0] = ht_t
                load_ht_head(r0, hts[r0])
                load_act_ht(r0, hts[r0])
            hp_t = hp_pool.tile([L, ROUND, H2], bf16, tag="hp")
            hps[0] = hp_t
            p0, p1 = HP_SPLIT[0], HP_SPLIT[0] + HP_SPLIT[1]
            load_hp(nc.sync, 0, hps[0], 0, p0)
            load_hp(nc.scalar, 0, hps[0], p0, p1)
            load_hp(nc.gpsimd, 0, hps[0], p1, ROUND)
            for _ in gen0:  # entity-0 chain runs under the round-0 loads
                pass
            gen1 = None
            pending = None
            for ridx in range(NR):
                r1, r2 = ridx + 1, ridx + 2
                if r2 < NR:
                    ht_t = ht_pool.tile([128, NCH, ROUND * L], fp8, tag="hT")
                    hts[r2] = ht_t
                    load_ht_head(r2, hts[r2])
                if r1 < NR:
                    hp_t = hp_pool.tile([L, ROUND, H2], bf16, tag="hp")
                    hps[r1] = hp_t
                    load_hp(nc.sync, r1, hps[r1], 0, HP_SPLIT[0])
                if ridx == 1:
                    gen1 = entity_block(1, ent1)
                if ridx == 4 and gen1 is not None:
                    for _ in gen1:
                        pass
                    gen1 = None
                ent = ent0 if ridx < 4 else ent1
                fin = None
                if pending is not None:
                    prv = pending
                    fin = lambda p=prv, r=ridx - 1: finish_round(r, *p)
                state = emit_groups(
                    ridx, hps[ridx], hts[ridx], lambda e=ent: e["u2sb"],
                    drain=gen1,
                    nxt=(hps.get(r1), hts.get(r2), r1, r2),
                    fin=fin, last=(LAST_HALVES and ridx == NR - 1),
                )
                pending = state
            if not LAST_HALVES:
                emit_scores(*carry[0])
                carry[0] = None
                finish_round(NR - 1, *pending)
            # SP is idle at the end; es ships there, parallel to the z store
            nc.sync.dma_start(out=es_d, in_=esall[:, :])

    nc.compile()
    return nc


def _get_nc():
    if "nc" not in _CACHE:
        _CACHE["nc"] = _build_bass()
    return _CACHE["nc"]


def _to_bf16(x):
    import ml_dtypes
    return np.asarray(x, dtype=np.float32).astype(ml_dtypes.bfloat16)


def _to_fp8(x):
    import ml_dtypes
    return np.asarray(x, dtype=np.float32).astype(ml_dtypes.float8_e4m3)


def _prep_weights(inputs):
    """Host-side weight transposition/padding into the chunk layouts."""
    w_hid = np.asarray(inputs["W_hid"], dtype=np.float32)   # [50, 700]
    w_ent = np.asarray(inputs["W_ent"], dtype=np.float32)   # [50, 2400]
    lt = np.asarray(inputs["latent_types"], dtype=np.float32)  # [3, 600]
    v = np.asarray(inputs["v"], dtype=np.float32)           # [50, 1]

    # whidT [128, 6, 64]: chunks 0-4 = hidden features, chunk 5 = pos;
    # output columns padded 50 -> 64 (DoubleRow needs M in {64, 128})
    whidT = np.zeros((128, NCH, 64), np.float32)
    wf = w_hid.T  # [700, 50]
    for c in range(HCH):
        cw = min(128, H2 - c * 128)
        whidT[0:cw, c, 0:A] = wf[c * 128:c * 128 + cw]
    whidT[0:POSF, 5, 0:A] = wf[H2:H2 + POSF]

    # wentT [128, 20, 50]: quarter q (e1, e1t, e2, e2t), chunk c of 640-pad
    wentT = np.zeros((128, 4 * ECH, A), np.float32)
    we = w_ent.T  # [2400, 50]
    for q in range(4):
        for c in range(ECH):
            lo = q * H2 + c * 128
            cw = min(128, (q + 1) * H2 - lo)
            if cw > 0:
                wentT[0:cw, q * ECH + c, :] = we[lo:lo + cw]

    # ltT [128, 5, 3] transposed latent type chunks
    ltT = np.zeros((128, ECH, T), np.float32)
    ltf = lt.T  # [600, 3]
    for c in range(ECH):
        cw = min(128, H2 - c * 128)
        ltT[0:cw, c, :] = ltf[c * 128:c * 128 + cw]

    # v replicated at partition offsets 0 and 64 (paired-group scores)
    v128 = np.zeros((128, 1), np.float32)
    v128[0:A] = v
    v128[64:64 + A] = v

    return {
        "whidT": _to_fp8(whidT),
        "wentT": _to_bf16(wentT),
        "ltT": _to_bf16(ltT),
        "lt16": _to_bf16(lt),
        "v128": _to_bf16(v128),
    }


def make_in_maps(inputs):
    import ml_dtypes
    hidden16 = _to_bf16(inputs["hidden"])                    # [B, L, 600]
    hid_f = np.asarray(inputs["hidden"], np.float32)
    # ht8 [128, 6, B, L]: feature-major fp8 hidden chunks + pos chunk 5
    ht8 = np.zeros((128, NCH, B, L), ml_dtypes.float8_e4m3)
    hfT = hid_f.transpose(2, 0, 1)                           # [600, B, L]
    for c in range(HCH):
        cw = min(128, H2 - c * 128)
        ht8[0:cw, c] = hfT[c * 128:c * 128 + cw].astype(ml_dtypes.float8_e4m3)
    pos = np.concatenate(
        [np.asarray(inputs["pos1_emb"], np.float32),
         np.asarray(inputs["pos2_emb"], np.float32)], axis=2
    )                                                        # [B, L, 100]
    ht8[0:POSF, 5] = pos.transpose(2, 0, 1).astype(ml_dtypes.float8_e4m3)

    e1 = np.asarray(inputs["entity1_idx"]).astype(np.int64)
    e2 = np.asarray(inputs["entity2_idx"]).astype(np.int64)
    weights = _prep_weights(inputs)

    loc = np.arange(BC, dtype=np.int64)
    in_maps = []
    for c in range(NCORES):
        sl = slice(c * BC, (c + 1) * BC)
        # hid_l: l-major [L, BC, H2]; gather rows index (l * BC + i)
        hid_l = np.ascontiguousarray(hidden16[sl].transpose(1, 0, 2))
        # ht8r: round-major [NR, 128, NCH * ROUND * L]
        ht8r = np.ascontiguousarray(
            ht8[:, :, sl, :].reshape(128, NCH, NR, ROUND * L)
            .transpose(2, 0, 1, 3)).reshape(NR, 128, NCH * ROUND * L)
        in_maps.append({
            "hid_l": hid_l,
            "ht8r": ht8r,
            "e1rows": np.ascontiguousarray(
                (e1[sl] * BC + loc).astype(np.int32)[:, None]),
            "e2rows": np.ascontiguousarray(
                (e2[sl] * BC + loc).astype(np.int32)[:, None]),
            **weights,
        })
    return in_maps


def unshard_z(zt, es):
    # zt: [NR//2, 128, ECH, 2, ROUND] with
    #   z[(2*pair + s)*ROUND + q, c*128 + p] = zt[pair, p, c, s, q]
    # except chunk 4 holds features 472:600 (overlaps chunk 3)
    z = np.transpose(np.asarray(zt, dtype=np.float32), (0, 3, 4, 2, 1))
    z = z.reshape(BC, ECH * 128)
    z = np.concatenate([z[:, 0:512], z[:, 512 + 40:640]], axis=1)
    # es: [ROUND+16, NR]; batch r*ROUND+q -> es[q, r], except the last
    # round's second half which lands in alias rows 32:48 when the last
    # round is finished in halves
    es = np.asarray(es, dtype=np.float32).copy()
    if LAST_HALVES:
        es[16:ROUND, NR - 1] = es[ROUND:ROUND + 16, NR - 1]
    den = es[:ROUND].T.reshape(BC, 1)
    return z / den


def kernel(**inputs):
    from concourse.bass_utils import run_bass_kernel_spmd

    nc = _get_nc()
    in_maps = make_in_maps(inputs)
    res = None
    for attempt in range(3):
        try:
            res = run_bass_kernel_spmd(
                nc, in_maps, core_ids=list(range(NCORES)))
            break
        except Exception:
            # the axon transport occasionally drops a run; retry
            if attempt == 2:
                raise
    _CACHE["last_res"] = res
    outs = [unshard_z(r["z"], r["esum"]) for r in res.results]
    return np.concatenate(outs, axis=0).astype(np.float32)


# revision 93
# speedup vs baseline: 1.0642x; 1.0642x over previous
"""EntityAwareAttention Trainium2 kernel, v3.

Per batch b of B=2048:
    hid_e{1,2} = hidden[b, e{1,2}_idx[b]]                       # [600]
    e{1,2}_type = softmax(hid_e @ LT.T) @ LT                    # [600], T=3
    u1 = concat(hidden, pos1, pos2) @ W_hid.T                   # [128, 50]
    u2 = concat(hid_e1, e1_type, hid_e2, e2_type) @ W_ent.T     # [50]
    u = tanh(u1 + u2); scores = u @ v; alpha = softmax(scores)  # [128]
    z = alpha @ hidden[b]                                       # [600]

Pure data parallel over batch: 8 cores x 256 batches, weights replicated.

v3 design (~2x the v2 kernel on the CoreSim cost model):
  - v2's bottleneck was PSUM evacuation of on-chip PE transposes
    (DVE 87% / Act 85% busy, nearly all tensor-copy).  v3 deletes the
    transpose pipeline entirely: the host pre-packs hidden a second
    time in feature-major fp8 (ht8 [128, 6ch, BC*L], pos folded in as
    chunk 5), DMA'd straight into the u1 rhs slot.  Token-major bf16
    hidden is still loaded for the z path (z matmuls have free-size-1
    outputs, which the PE does at negligible cost).
  - u1 matmuls in fp8 DoubleRow, group pairs stacked vertically in one
    PSUM bank (rows 0:64 / 64:128) so a single tanh covers 8 batches;
    v is host-replicated to partitions 64-113 so the per-batch score
    matmuls can read either half.
  - Scores lag one group pair (carried across rounds) so the PE never
    waits on tanh; softmax is unnormalized (host divides); z goes
    PSUM -> DRAM directly, batched 2 rounds per DMA; esum accumulates
    on-chip all 8 rounds and ships once.
  - DMA is the cost floor (hidden 1.5 copies + pos ~ 24.3us/round of
    queue time) and only SP/Act/Pool can issue DMAs, so loads are
    split SP: 3 hT chunks + 9 hp batches, Act: 1 + 11 (Act also runs
    tanh/exp), Pool: 2 + 12 (+ gathers and stores).  Entity/u2 chain
    unchanged from v2 except PSUM evacs moved Act -> DVE (DVE is
    otherwise idle; Act is a DMA queue now).
"""

import numpy as np

B, L, H2, PP, A, T = 2048, 128, 600, 50, 50, 3
NCORES = 8
BC = B // NCORES   # 256 batches per core
SB = 128           # superbatch for the entity/u2 pipeline
ROUND = 32         # batches per round
GROUP = 4          # batches per u1 matmul group (N = 4*128 = 512)
NPAIR = ROUND // (2 * GROUP)  # group pairs per round
NR = BC // ROUND   # rounds per core
NCH = 6            # rhs feature chunks (5 hidden + 1 pos)
HCH = 5            # hidden chunks (4x128 + 88)
EPAD = 640         # entity vectors padded to 5x128
ECH = 5
POSF = 2 * PP      # 100 pos features

# DMA queue split for the per-round loads (SP / Act / Pool)
HT_SPLIT = (3, 1, 2)    # of the 6 ht8 chunks
HP_SPLIT = (10, 10, 12)  # of the 32 hp batches
LAST_HALVES = True      # finish the last round in halves (shorter tail)

_CACHE = {}


def _build_bass():
    import concourse.bass as bass
    import concourse.bacc as bacc
    import concourse.tile as tile
    from concourse import mybir
    from concourse.masks import make_identity

    f32 = mybir.dt.float32
    bf16 = mybir.dt.bfloat16
    fp8 = mybir.dt.float8e4
    i32 = mybir.dt.int32
    AF = mybir.ActivationFunctionType
    AX = mybir.AxisListType
    DR = mybir.MatmulPerfMode.DoubleRow

    nc = bacc.Bacc("TRN2", debug=False, target_bir_lowering=False)

    # hidden is staged l-major ([L, BC, H2]) and ht8 round-major so each
    # round's load is one long contiguous run per partition
    hid_d = nc.dram_tensor("hid_l", [L, BC, H2], bf16, kind="ExternalInput").ap()
    ht8_d = nc.dram_tensor(
        "ht8r", [NR, 128, NCH * ROUND * L], fp8, kind="ExternalInput").ap()
    e1r_d = nc.dram_tensor("e1rows", [BC, 1], i32, kind="ExternalInput").ap()
    e2r_d = nc.dram_tensor("e2rows", [BC, 1], i32, kind="ExternalInput").ap()
    # host-pretransposed weights
    whidT_d = nc.dram_tensor("whidT", [128, NCH, 64], fp8, kind="ExternalInput").ap()
    wentT_d = nc.dram_tensor("wentT", [128, 4 * ECH, A], bf16, kind="ExternalInput").ap()
    ltT_d = nc.dram_tensor("ltT", [128, ECH, T], bf16, kind="ExternalInput").ap()
    lt16_d = nc.dram_tensor("lt16", [T, H2], bf16, kind="ExternalInput").ap()
    v_d = nc.dram_tensor("v128", [128, 1], bf16, kind="ExternalInput").ap()
    z_d = nc.dram_tensor(
        "z", [NR // 2, 128, ECH, 2, ROUND], f32, kind="ExternalOutput"
    ).ap()
    # per-batch softmax denominators; z is stored unnormalized and the
    # host divides (keeps the recip/scale off the round critical path)
    # rows 32:48 alias the last round's second half (a partition-start-16
    # write is not ISA-legal, so it lands at partition 32 instead)
    es_d = nc.dram_tensor(
        "esum", [ROUND + 16, NR], f32, kind="ExternalOutput").ap()

    hid_flat = hid_d.rearrange("l b d -> (l b) d")

    with tile.TileContext(nc) as tc, nc.allow_non_contiguous_dma(
        "partition-dim-last APs for the parallel DMA fabric"
    ):
        with (
            tc.tile_pool(name="const", bufs=1) as const,
            tc.tile_pool(name="hp_pool", bufs=3) as hp_pool,
            tc.tile_pool(name="ht_pool", bufs=3) as ht_pool,
            tc.tile_pool(name="u_pool", bufs=2) as u_pool,
            tc.tile_pool(name="ent_pool", bufs=2) as ent_pool,
            tc.tile_pool(name="small", bufs=4) as small,
            tc.tile_pool(name="zs_pool", bufs=2) as zs_pool,
            tc.tile_pool(name="ps_u1", bufs=2, space="PSUM") as ps_u1,
            tc.tile_pool(name="ps_h", bufs=2, space="PSUM") as ps_h,
            tc.tile_pool(name="ps_sc", bufs=2, space="PSUM") as ps_sc,
            tc.tile_pool(name="ps_z", bufs=1, space="PSUM") as ps_z,
            tc.tile_pool(name="ps_misc", bufs=1, space="PSUM") as ps_misc,
        ):
            # ---------------- constants (all host-prepacked) ----------------
            id_f32 = const.tile([128, 128], f32)
            make_identity(nc, id_f32[:, :])
            id_bf = const.tile([128, 128], bf16)
            nc.vector.tensor_copy(id_bf[:, :], id_f32[:, :])

            # const loads spread across the three DMA queues so no single
            # queue delays the round-0 loads by the full preamble
            whidT = const.tile([128, NCH, 64], fp8)
            nc.sync.dma_start(out=whidT[:, :, :], in_=whidT_d)
            wentT = const.tile([128, 4 * ECH, A], bf16)
            nc.scalar.dma_start(out=wentT[:, :, :], in_=wentT_d)
            v128 = const.tile([128, 1], bf16)
            nc.scalar.dma_start(out=v128[:, :], in_=v_d)
            ltT = const.tile([128, ECH, T], bf16)
            nc.gpsimd.dma_start(out=ltT[:, :, :], in_=ltT_d)
            lt16 = const.tile([T, H2], bf16)
            nc.gpsimd.dma_start(out=lt16[:, :], in_=lt16_d)
            # preload the activation table while DMAs warm up (one table
            # serves Tanh and Exp; otherwise the 1.3us load lands in the
            # middle of the entity chain)
            atl = const.tile([1, 1], f32)
            nc.scalar.activation(atl[:, :], id_f32[0:1, 0:1], AF.Tanh)
            esall = const.tile([ROUND + 16, NR], f32)
            nc.gpsimd.memset(esall[:, :], 0.0)  # rows 16:32 of the last
            # column are dead (their halves land in alias rows 32:48)

            def entity_block(s, out):
                """Gather + latent-type + u2 for superbatch s (128 batches).
                Generator: yields between cross-engine stages so the driver
                can interleave them with round groups (keeps the serial
                chain out of PE's in-order queue).  Stores the u2 tile in
                out["u2sb"]."""
                srcT = []
                tiles = []
                for rows_d in (e1r_d, e2r_d):
                    rows = ent_pool.tile([SB, 1], i32, tag="rows")
                    nc.gpsimd.dma_start(
                        out=rows[:, :], in_=rows_d[s * SB:(s + 1) * SB, :]
                    )
                    ent = ent_pool.tile([SB, H2], bf16, tag="ent")
                    nc.gpsimd.indirect_dma_start(
                        out=ent[:, 0:H2],
                        out_offset=None,
                        in_=hid_flat,
                        in_offset=bass.IndirectOffsetOnAxis(ap=rows[:, 0:1], axis=0),
                    )
                    tiles.append(ent)
                yield
                # chunk 4 is 88 wide (600 = 4*128 + 88); no pad, no memset
                CW = [min(128, H2 - c * 128) for c in range(ECH)]
                for ent in tiles:
                    entT = ent_pool.tile([128, ECH, SB], bf16, tag="entT")
                    tp = ps_misc.tile([128, ECH, SB], bf16, tag="misc")
                    for c in range(ECH):
                        nc.tensor.transpose(
                            tp[0:CW[c], c, :],
                            ent[:, c * 128:c * 128 + CW[c]], id_bf[:, :]
                        )
                    nc.vector.tensor_copy(entT[:, 0:4, :], tp[:, 0:4, :])
                    nc.vector.tensor_copy(entT[0:CW[4], 4, :], tp[0:CW[4], 4, :])
                    yield
                    # latent-type logits [3, 128]
                    lg_ps = ps_misc.tile([T, SB], f32, tag="misc")
                    for c in range(ECH):
                        nc.tensor.matmul(
                            lg_ps[:, :], lhsT=ltT[0:CW[c], c, :],
                            rhs=entT[0:CW[c], c, :],
                            start=(c == 0), stop=(c == ECH - 1),
                        )
                    lgT_sb = ent_pool.tile([T, SB], f32, tag="lgT")
                    nc.vector.tensor_copy(lgT_sb[:, :], lg_ps[:, :])
                    yield
                    lg2_ps = ps_misc.tile([SB, T], f32, tag="misc")
                    nc.tensor.transpose(lg2_ps[:, :], lgT_sb[:, :], id_f32[0:T, 0:T])
                    expl = ent_pool.tile([SB, T], f32, tag="expl")
                    nc.scalar.activation(expl[:, :], lg2_ps[:, :], AF.Exp)
                    yield
                    ssum = ent_pool.tile([SB, 1], f32, tag="ssum")
                    nc.vector.reduce_sum(ssum[:, :], expl[:, :], axis=AX.X)
                    srec = ent_pool.tile([SB, 1], f32, tag="srec")
                    nc.vector.reciprocal(srec[:, :], ssum[:, :])
                    attw = ent_pool.tile([SB, T], f32, tag="attw")
                    nc.vector.tensor_scalar_mul(attw[:, :], expl[:, :], srec[:, 0:1])
                    yield
                    awT_ps = ps_misc.tile([T, SB], f32, tag="misc")
                    nc.tensor.transpose(awT_ps[:, :], attw[:, :], id_f32[:, :])
                    awT = ent_pool.tile([T, SB], bf16, tag="awT_sb")
                    nc.vector.tensor_copy(awT[:, :], awT_ps[:, :])
                    yield
                    # e_type = attw @ LT : [128, 600]
                    et = ent_pool.tile([SB, H2], bf16, tag="et_sb")
                    et_lo = ps_misc.tile([SB, 512], f32, tag="misc")
                    nc.tensor.matmul(
                        et_lo[:, :], lhsT=awT[:, :], rhs=lt16[:, 0:512],
                        start=True, stop=True,
                    )
                    nc.vector.tensor_copy(et[:, 0:512], et_lo[:, :])
                    yield
                    et_hi = ps_misc.tile([SB, 128], f32, tag="misc")
                    nc.tensor.matmul(
                        et_hi[:, 0:H2 - 512], lhsT=awT[:, :], rhs=lt16[:, 512:H2],
                        start=True, stop=True,
                    )
                    nc.vector.tensor_copy(et[:, 512:H2], et_hi[:, 0:H2 - 512])
                    yield
                    etT = ent_pool.tile([128, ECH, SB], bf16, tag="etT")
                    tp2 = ps_misc.tile([128, ECH, SB], bf16, tag="misc")
                    for c in range(ECH):
                        nc.tensor.transpose(
                            tp2[0:CW[c], c, :],
                            et[:, c * 128:c * 128 + CW[c]], id_bf[:, :]
                        )
                    nc.vector.tensor_copy(etT[:, 0:4, :], tp2[:, 0:4, :])
                    nc.vector.tensor_copy(etT[0:CW[4], 4, :], tp2[0:CW[4], 4, :])
                    yield
                    srcT.append((entT, etT))

                u2_ps = ps_misc.tile([A, SB], f32, tag="misc")
                order = [srcT[0][0], srcT[0][1], srcT[1][0], srcT[1][1]]
                k = 0
                for q in range(4):
                    for c in range(ECH):
                        cw = min(128, H2 - c * 128)
                        nc.tensor.matmul(
                            u2_ps[:, :],
                            lhsT=wentT[0:cw, q * ECH + c, :],
                            rhs=order[q][0:cw, c, :],
                            start=(k == 0), stop=(k == 19),
                        )
                        k += 1
                # padded to 64 rows (rows 50:64 zero) so the fused
                # relocate+u2-add and the id-matmul can cover the DR pad
                u2sb = ent_pool.tile([64, SB], bf16, tag="u2sb")
                nc.gpsimd.memset(u2sb[32:64, :], 0.0)
                nc.vector.tensor_copy(u2sb[0:A, :], u2_ps[:, :])
                out["u2sb"] = u2sb

            def load_hp(eng, ridx, hp, a, b):
                b0 = ridx * ROUND
                eng.dma_start(
                    out=hp[:, a:b, :],
                    in_=hid_d[:, b0 + a:b0 + b, :],
                )

            def emit_scores(sc_ps, pr, uT):
                for j in range(2 * GROUP):
                    half, jj = divmod(j, GROUP)
                    off = 64 * half
                    bl = pr * 2 * GROUP + j
                    nc.tensor.matmul(
                        sc_ps[:, bl:bl + 1],
                        lhsT=uT[off:off + A, jj * L:(jj + 1) * L],
                        rhs=v128[off:off + A, 0:1],
                        start=True, stop=True,
                    )

            carry = [None]  # (sc_ps, pair, uT) with scores not yet emitted

            def emit_groups(ridx, hp, hT, u2sb_fn, drain=None, nxt=None,
                            fin=None, last=False):
                """u1 + tanh for round ridx; group pairs share one PSUM bank
                (rows 0:64 / 64:128) so one tanh covers 8 batches.  Scores
                lag one pair, carried across rounds."""
                s, r = divmod(ridx, SB // ROUND)
                sc_ps = ps_sc.tile([L, ROUND], f32, tag="scT")
                for pr in range(NPAIR):
                    # group pair stacked on partitions (rows 0:64 / 64:128).
                    # The ISA requires matmul dst partition 0, so the odd
                    # group lands in a scratch bank and the otherwise-idle
                    # DVE relocates it — fusing in that group's u2 add for
                    # free; one tanh then covers 8 batches.
                    u1_ps = ps_u1.tile([128, GROUP * L], f32, tag="u1like")
                    hb_ps = ps_h.tile([64, GROUP * L], f32, tag="u1hi")
                    u2sb16 = u2sb_fn()
                    for half in range(2):
                        g = 2 * pr + half
                        dst = u1_ps if half == 0 else hb_ps
                        gsl = slice(g * GROUP * L, (g + 1) * GROUP * L)
                        for c in range(3):
                            nc.tensor.matmul(
                                dst[0:64, :],
                                lhsT=whidT[:, 2 * c:2 * c + 2, :],
                                rhs=hT[:, 2 * c:2 * c + 2, gsl],
                                start=(c == 0),
                                stop=(c == 2 and half == 1),
                                perf_mode=DR, skip_group_check=True,
                            )
                        b0r = r * ROUND + g * GROUP
                        u2r = u2sb16[0:64, b0r:b0r + GROUP]
                        u2b = bass.AP(
                            tensor=u2r.tensor, offset=u2r.offset,
                            ap=[u2r.ap[0], u2r.ap[1], [0, L]],
                        )
                        if half == 0:
                            # += u2 broadcast over tokens via identity matmul
                            nc.tensor.matmul(
                                u1_ps[0:64, :], lhsT=id_bf[0:64, 0:64],
                                rhs=u2b, start=False, stop=True,
                                skip_group_check=True,
                            )
                        else:
                            # relocate + u2 add in one DVE pass
                            nc.vector.scalar_tensor_tensor(
                                u1_ps[64:128, :].rearrange(
                                    "p (i l) -> p i l", i=GROUP),
                                hb_ps[:, :].rearrange(
                                    "p (i l) -> p i l", i=GROUP),
                                0.0, u2b,
                                op0=mybir.AluOpType.bypass,
                                op1=mybir.AluOpType.add,
                            )
                    uT = u_pool.tile([128, GROUP * L], bf16, tag="uT")
                    nc.scalar.activation(uT[:, :], u1_ps[:, :], AF.Tanh)
                    if drain is not None:
                        next(drain, None)
                        next(drain, None)
                    if nxt is not None:
                        # next rounds' Act/Pool loads, sliced between tanhs
                        # (hT prefetches two rounds ahead, hp one)
                        hp1, hT2, r1, r2 = nxt
                        p0, p1 = HP_SPLIT[0], HP_SPLIT[0] + HP_SPLIT[1]
                        mid = (p0 + p1) // 2
                        pm = (p1 + ROUND) // 2
                        if pr == 0:
                            if hT2 is not None:
                                load_act_ht(r2, hT2)
                        elif pr == 1:
                            if hp1 is not None:
                                load_hp(nc.scalar, r1, hp1, p0, mid)
                                load_hp(nc.gpsimd, r1, hp1, p1, pm)
                        elif pr == 2:
                            if hp1 is not None:
                                load_hp(nc.scalar, r1, hp1, mid, p1)
                                load_hp(nc.gpsimd, r1, hp1, pm, ROUND)
                    if carry[0] is not None:
                        emit_scores(*carry[0])
                        carry[0] = None
                    if last:
                        # no score lag in the last round: finish in halves
                        # as the scores become available to shorten the tail
                        emit_scores(sc_ps, pr, uT)
                        if pr == 1:
                            if fin is not None:
                                fin()
                            finish_round(ridx, hp, sc_ps, 0, ROUND // 2)
                        elif pr == NPAIR - 1:
                            finish_round(ridx, hp, sc_ps, ROUND // 2, ROUND)
                    else:
                        carry[0] = (sc_ps, pr, uT)
                        if pr == 1 and fin is not None:
                            fin()
                return hp, sc_ps

            zcur = [None, None]  # [zt_sb pair tile, zt_ps round tile]

            def finish_round(ridx, hp, sc_ps, q0=0, q1=ROUND):
                """Softmax numerator + z for batches q0:q1 of one round.
                Normally emitted (whole round) inside the NEXT round's
                group stream; the last round is finished in halves to
                shorten the serial tail.  z accumulates in PSUM across a
                round pair and ships one DMA per pair."""
                zslot = ridx % 2
                if q0 == 0:
                    if zslot == 0:
                        zsb_new = zs_pool.tile(
                            [128, ECH, 2, ROUND], f32, tag="zt_sb")
                        zcur[0] = zsb_new
                    zps_new = ps_z.tile([128, ECH, ROUND], f32, tag="zt")
                    zcur[1] = zps_new
                zt_sb, zt_ps = zcur
                n = q1 - q0
                scT_sb = small.tile([L, n], bf16, tag="scT_sb")
                nc.vector.tensor_copy(scT_sb[:, :], sc_ps[:, q0:q1])
                sc2_ps = ps_misc.tile([n, L], bf16, tag="misc")
                nc.tensor.transpose(sc2_ps[:, :], scT_sb[:, :], id_bf[:, :])
                exps = small.tile([n, L], bf16, tag="exps")
                nc.scalar.activation(exps[:, :], sc2_ps[:, :], AF.Exp)
                er0 = q0 if q0 % 32 == 0 else 32
                nc.vector.reduce_sum(esall[er0:er0 + n, ridx:ridx + 1],
                                     exps[:, :], axis=AX.X)
                aT_ps = ps_misc.tile([L, n], bf16, tag="misc")
                nc.tensor.transpose(aT_ps[:, :], exps[:, :], id_bf[0:n, 0:n])
                alphaT = small.tile([L, n], bf16, tag="alphaT")
                nc.vector.tensor_copy(alphaT[:, :], aT_ps[:, :])

                # zT[d, b] = sum_l hp[l, b, d] * exps[l, b]  (unnormalized)
                # chunk 4 covers features 472:600 (overlapping chunk 3) so
                # every PSUM row is written; the host drops the overlap
                for q in range(q0, q1):
                    for c in range(HCH):
                        oc = c * 128 if c < 4 else H2 - 128
                        nc.tensor.matmul(
                            zt_ps[:, c, q:q + 1],
                            lhsT=hp[:, q, oc:oc + 128],
                            rhs=alphaT[:, q - q0:q - q0 + 1],
                            start=True, stop=True,
                        )
                nc.vector.tensor_copy(
                    zt_sb[:, :, zslot, q0:q1], zt_ps[:, :, q0:q1])
                if zslot == 1 and q1 == ROUND:
                    nc.gpsimd.dma_start(
                        out=z_d[ridx // 2], in_=zt_sb[:, :, :, :]
                    )

            def ht_dma(eng, ridx, hT, ca, cb):
                eng.dma_start(
                    out=hT[:, ca:cb, :],
                    in_=ht8_d[ridx][:, ca * ROUND * L:cb * ROUND * L].rearrange(
                        "p (c n) -> p c n", c=cb - ca),
                )

            def load_ht_head(ridx, hT):
                """SP + Pool shares of round ridx's hT chunks."""
                c0, c1 = HT_SPLIT[0], HT_SPLIT[0] + HT_SPLIT[1]
                ht_dma(nc.sync, ridx, hT, 0, c0)
                ht_dma(nc.gpsimd, ridx, hT, c1, NCH)

            def load_act_ht(ridx, hT):
                c0, c1 = HT_SPLIT[0], HT_SPLIT[0] + HT_SPLIT[1]
                ht_dma(nc.scalar, ridx, hT, c0, c1)

            # ---------------- main schedule ----------------
            ent0, ent1 = {}, {}
            gen0 = entity_block(0, ent0)
            next(gen0)  # issue the gathers before anything else
            # hT is prefetched two rounds deep (it gates each round's start);
            # hp only one (first read a round and a half later, by z)
            hts = {}
            hps = {}
            for r0 in range(2):
                ht_t = ht_pool.tile([128, NCH, ROUND * L], fp8, tag="hT")
                hts[r0] = ht_t
                load_ht_head(r0, hts[r0])
                load_act_ht(r0, hts[r0])
            hp_t = hp_pool.tile([L, ROUND, H2], bf16, tag="hp")
            hps[0] = hp_t
            p0, p1 = HP_SPLIT[0], HP_SPLIT[0] + HP_SPLIT[1]
            load_hp(nc.sync, 0, hps[0], 0, p0)
            load_hp(nc.scalar, 0, hps[0], p0, p1)
            load_hp(nc.gpsimd, 0, hps[0], p1, ROUND)
            for _ in gen0:  # entity-0 chain runs under the round-0 loads
                pass
            gen1 = None
            pending = None
            for ridx in range(NR):
                r1, r2 = ridx + 1, ridx + 2
                if r2 < NR:
                    ht_t = ht_pool.tile([128, NCH, ROUND * L], fp8, tag="hT")
                    hts[r2] = ht_t
                    load_ht_head(r2, hts[r2])
                if r1 < NR:
                    hp_t = hp_pool.tile([L, ROUND, H2], bf16, tag="hp")
                    hps[r1] = hp_t
                    load_hp(nc.sync, r1, hps[r1], 0, HP_SPLIT[0])
                if ridx == 1:
                    gen1 = entity_block(1, ent1)
                if ridx == 4 and gen1 is not None:
                    for _ in gen1:
                        pass
                    gen1 = None
                ent = ent0 if ridx < 4 else ent1
                fin = None
                if pending is not None:
                    prv = pending
                    fin = lambda p=prv, r=ridx - 1: finish_round(r, *p)
                state = emit_groups(
                    ridx, hps[ridx], hts[ridx], lambda e=ent: e["u2sb"],
                    drain=gen1,
                    nxt=(hps.get(r1), hts.get(r2), r1, r2),
                    fin=fin, last=(LAST_HALVES and ridx == NR - 1),
                )
                pending = state
            if not LAST_HALVES:
                emit_scores(*carry[0])
                carry[0] = None
                finish_round(NR - 1, *pending)
            # SP is idle at the end; es ships there, parallel to the z store
            nc.sync.dma_start(out=es_d, in_=esall[:, :])

    nc.compile()
    return nc


def _get_nc():
    if "nc" not in _CACHE:
        _CACHE["nc"] = _build_bass()
    return _CACHE["nc"]


def _to_bf16(x):
    import ml_dtypes
    return np.asarray(x, dtype=np.float32).astype(ml_dtypes.bfloat16)


def _to_fp8(x):
    import ml_dtypes
    return np.asarray(x, dtype=np.float32).astype(ml_dtypes.float8_e4m3)


def _prep_weights(inputs):
    """Host-side weight transposition/padding into the chunk layouts."""
    w_hid = np.asarray(inputs["W_hid"], dtype=np.float32)   # [50, 700]
    w_ent = np.asarray(inputs["W_ent"], dtype=np.float32)   # [50, 2400]
    lt = np.asarray(inputs["latent_types"], dtype=np.float32)  # [3, 600]
    v = np.asarray(inputs["v"], dtype=np.float32)           # [50, 1]

    # whidT [128, 6, 64]: chunks 0-4 = hidden features, chunk 5 = pos;
    # output columns padded 50 -> 64 (DoubleRow needs M in {64, 128})
    whidT = np.zeros((128, NCH, 64), np.float32)
    wf = w_hid.T  # [700, 50]
    for c in range(HCH):
        cw = min(128, H2 - c * 128)
        whidT[0:cw, c, 0:A] = wf[c * 128:c * 128 + cw]
    whidT[0:POSF, 5, 0:A] = wf[H2:H2 + POSF]

    # wentT [128, 20, 50]: quarter q (e1, e1t, e2, e2t), chunk c of 640-pad
    wentT = np.zeros((128, 4 * ECH, A), np.float32)
    we = w_ent.T  # [2400, 50]
    for q in range(4):
        for c in range(ECH):
            lo = q * H2 + c * 128
            cw = min(128, (q + 1) * H2 - lo)
            if cw > 0:
                wentT[0:cw, q * ECH + c, :] = we[lo:lo + cw]

    # ltT [128, 5, 3] transposed latent type chunks
    ltT = np.zeros((128, ECH, T), np.float32)
    ltf = lt.T  # [600, 3]
    for c in range(ECH):
        cw = min(128, H2 - c * 128)
        ltT[0:cw, c, :] = ltf[c * 128:c * 128 + cw]

    # v replicated at partition offsets 0 and 64 (paired-group scores)
    v128 = np.zeros((128, 1), np.float32)
    v128[0:A] = v
    v128[64:64 + A] = v

    return {
        "whidT": _to_fp8(whidT),
        "wentT": _to_bf16(wentT),
        "ltT": _to_bf16(ltT),
        "lt16": _to_bf16(lt),
        "v128": _to_bf16(v128),
    }


def make_in_maps(inputs):
    import ml_dtypes
    hidden16 = _to_bf16(inputs["hidden"])                    # [B, L, 600]
    hid_f = np.asarray(inputs["hidden"], np.float32)
    # ht8 [128, 6, B, L]: feature-major fp8 hidden chunks + pos chunk 5
    ht8 = np.zeros((128, NCH, B, L), ml_dtypes.float8_e4m3)
    hfT = hid_f.transpose(2, 0, 1)                           # [600, B, L]
    for c in range(HCH):
        cw = min(128, H2 - c * 128)
        ht8[0:cw, c] = hfT[c * 128:c * 128 + cw].astype(ml_dtypes.float8_e4m3)
    pos = np.concatenate(
        [np.asarray(inputs["pos1_emb"], np.float32),
         np.asarray(inputs["pos2_emb"], np.float32)], axis=2
    )                                                        # [B, L, 100]
    ht8[0:POSF, 5] = pos.transpose(2, 0, 1).astype(ml_dtypes.float8_e4m3)

    e1 = np.asarray(inputs["entity1_idx"]).astype(np.int64)
    e2 = np.asarray(inputs["entity2_idx"]).astype(np.int64)
    weights = _prep_weights(inputs)

    loc = np.arange(BC, dtype=np.int64)
    in_maps = []
    for c in range(NCORES):
        sl = slice(c * BC, (c + 1) * BC)
        # hid_l: l-major [L, BC, H2]; gather rows index (l * BC + i)
        hid_l = np.ascontiguousarray(hidden16[sl].transpose(1, 0, 2))
        # ht8r: round-major [NR, 128, NCH * ROUND * L]
        ht8r = np.ascontiguousarray(
            ht8[:, :, sl, :].reshape(128, NCH, NR, ROUND * L)
            .transpose(2, 0, 1, 3)).reshape(NR, 128, NCH * ROUND * L)
        in_maps.append({
            "hid_l": hid_l,
            "ht8r": ht8r,
            "e1rows": np.ascontiguousarray(
                (e1[sl] * BC + loc).astype(np.int32)[:, None]),
            "e2rows": np.ascontiguousarray(
                (e2[sl] * BC + loc).astype(np.int32)[:, None]),
            **weights,
        })
    return in_maps


def unshard_z(zt, es):
    # zt: [NR//2, 128, ECH, 2, ROUND] with
    #   z[(2*pair + s)*ROUND + q, c*128 + p] = zt[pair, p, c, s, q]
    # except chunk 4 holds features 472:600 (overlaps chunk 3)
    z = np.transpose(np.asarray(zt, dtype=np.float32), (0, 3, 4, 2, 1))
    z = z.reshape(BC, ECH * 128)
    z = np.concatenate([z[:, 0:512], z[:, 512 + 40:640]], axis=1)
    # es: [ROUND+16, NR]; batch r*ROUND+q -> es[q, r], except the last
    # round's second half which lands in alias rows 32:48 when the last
    # round is finished in halves
    es = np.asarray(es, dtype=np.float32).copy()
    if LAST_HALVES:
        es[16:ROUND, NR - 1] = es[ROUND:ROUND + 16, NR - 1]
    den = es[:ROUND].T.reshape(BC, 1)
    return z / den


def kernel(**inputs):
    from concourse.bass_utils import run_bass_kernel_spmd

    nc = _get_nc()
    in_maps = make_in_maps(inputs)
    res = None
    for attempt in range(3):
        try:
            res = run_bass_kernel_spmd(
                nc, in_maps, core_ids=list(range(NCORES)))
            break
        except Exception:
            # the axon transport occasionally drops a run; retry
            if attempt == 2:
                raise
    _CACHE["last_res"] = res
    outs = [unshard_z(r["z"], r["esum"]) for r in res.results]
    return np.concatenate(outs, axis=0).astype(np.float32)


# revision 97
# speedup vs baseline: 1.0840x; 1.0187x over previous
"""EntityAwareAttention Trainium2 kernel, v3.

Per batch b of B=2048:
    hid_e{1,2} = hidden[b, e{1,2}_idx[b]]                       # [600]
    e{1,2}_type = softmax(hid_e @ LT.T) @ LT                    # [600], T=3
    u1 = concat(hidden, pos1, pos2) @ W_hid.T                   # [128, 50]
    u2 = concat(hid_e1, e1_type, hid_e2, e2_type) @ W_ent.T     # [50]
    u = tanh(u1 + u2); scores = u @ v; alpha = softmax(scores)  # [128]
    z = alpha @ hidden[b]                                       # [600]

Pure data parallel over batch: 8 cores x 256 batches, weights replicated.

v3 design (~2x the v2 kernel on the CoreSim cost model):
  - v2's bottleneck was PSUM evacuation of on-chip PE transposes
    (DVE 87% / Act 85% busy, nearly all tensor-copy).  v3 deletes the
    transpose pipeline entirely: the host pre-packs hidden a second
    time in feature-major fp8 (ht8 [128, 6ch, BC*L], pos folded in as
    chunk 5), DMA'd straight into the u1 rhs slot.  Token-major bf16
    hidden is still loaded for the z path (z matmuls have free-size-1
    outputs, which the PE does at negligible cost).
  - u1 matmuls in fp8 DoubleRow, group pairs stacked vertically in one
    PSUM bank (rows 0:64 / 64:128) so a single tanh covers 8 batches;
    v is host-replicated to partitions 64-113 so the per-batch score
    matmuls can read either half.
  - Scores lag one group pair (carried across rounds) so the PE never
    waits on tanh; softmax is unnormalized (host divides); z goes
    PSUM -> DRAM directly, batched 2 rounds per DMA; esum accumulates
    on-chip all 8 rounds and ships once.
  - DMA is the cost floor (hidden 1.5 copies + pos ~ 24.3us/round of
    queue time) and only SP/Act/Pool can issue DMAs, so loads are
    split SP: 3 hT chunks + 9 hp batches, Act: 1 + 11 (Act also runs
    tanh/exp), Pool: 2 + 12 (+ gathers and stores).  Entity/u2 chain
    unchanged from v2 except PSUM evacs moved Act -> DVE (DVE is
    otherwise idle; Act is a DMA queue now).
"""

import numpy as np

B, L, H2, PP, A, T = 2048, 128, 600, 50, 50, 3
NCORES = 8
BC = B // NCORES   # 256 batches per core
SB = 128           # superbatch for the entity/u2 pipeline
ROUND = 32         # batches per round
GROUP = 4          # batches per u1 matmul group (N = 4*128 = 512)
NPAIR = ROUND // (2 * GROUP)  # group pairs per round
NR = BC // ROUND   # rounds per core
NCH = 6            # rhs feature chunks (5 hidden + 1 pos)
HCH = 5            # hidden chunks (4x128 + 88)
EPAD = 640         # entity vectors padded to 5x128
ECH = 5
POSF = 2 * PP      # 100 pos features

# DMA queue split for the per-round loads (SP / Act / Pool)
HT_SPLIT = (3, 1, 2)    # of the 6 ht8 chunks
HP_SPLIT = (10, 10, 12)  # of the 32 hp batches
LAST_HALVES = True      # finish the last round in halves (shorter tail)

_CACHE = {}


def _build_bass():
    import concourse.bass as bass
    import concourse.bacc as bacc
    import concourse.tile as tile
    from concourse import mybir
    from concourse.masks import make_identity

    f32 = mybir.dt.float32
    bf16 = mybir.dt.bfloat16
    fp8 = mybir.dt.float8e4
    i32 = mybir.dt.int32
    AF = mybir.ActivationFunctionType
    AX = mybir.AxisListType
    DR = mybir.MatmulPerfMode.DoubleRow

    nc = bacc.Bacc("TRN2", debug=False, target_bir_lowering=False)

    # hidden is staged l-major ([L, BC, H2]) and ht8 round-major so each
    # round's load is one long contiguous run per partition
    hid_d = nc.dram_tensor("hid_l", [L, BC, H2], bf16, kind="ExternalInput").ap()
    ht8_d = nc.dram_tensor(
        "ht8r", [NR, 128, NCH * ROUND * L], fp8, kind="ExternalInput").ap()
    # entity rows host-gathered (pure indexing, like the index math the
    # host already does): [BC, H2] bf16 each
    ent1_d = nc.dram_tensor("ent1", [BC, H2], bf16, kind="ExternalInput").ap()
    ent2_d = nc.dram_tensor("ent2", [BC, H2], bf16, kind="ExternalInput").ap()
    # host-pretransposed weights
    whidT_d = nc.dram_tensor("whidT", [128, NCH, 64], fp8, kind="ExternalInput").ap()
    wentT_d = nc.dram_tensor("wentT", [128, 4 * ECH, A], bf16, kind="ExternalInput").ap()
    ltT_d = nc.dram_tensor("ltT", [128, ECH, T], bf16, kind="ExternalInput").ap()
    lt16_d = nc.dram_tensor("lt16", [T, H2], bf16, kind="ExternalInput").ap()
    v_d = nc.dram_tensor("v128", [128, 1], bf16, kind="ExternalInput").ap()
    z_d = nc.dram_tensor(
        "z", [NR // 2, 128, ECH, 2, ROUND], f32, kind="ExternalOutput"
    ).ap()
    # per-batch softmax denominators; z is stored unnormalized and the
    # host divides (keeps the recip/scale off the round critical path)
    # rows 32:48 alias the last round's second half (a partition-start-16
    # write is not ISA-legal, so it lands at partition 32 instead)
    es_d = nc.dram_tensor(
        "esum", [ROUND + 16, NR], f32, kind="ExternalOutput").ap()

    with tile.TileContext(nc) as tc, nc.allow_non_contiguous_dma(
        "partition-dim-last APs for the parallel DMA fabric"
    ):
        with (
            tc.tile_pool(name="const", bufs=1) as const,
            tc.tile_pool(name="hp_pool", bufs=3) as hp_pool,
            tc.tile_pool(name="ht_pool", bufs=3) as ht_pool,
            tc.tile_pool(name="u_pool", bufs=2) as u_pool,
            tc.tile_pool(name="ent_pool", bufs=2) as ent_pool,
            tc.tile_pool(name="small", bufs=4) as small,
            tc.tile_pool(name="zs_pool", bufs=2) as zs_pool,
            tc.tile_pool(name="ps_u1", bufs=2, space="PSUM") as ps_u1,
            tc.tile_pool(name="ps_h", bufs=2, space="PSUM") as ps_h,
            tc.tile_pool(name="ps_sc", bufs=2, space="PSUM") as ps_sc,
            tc.tile_pool(name="ps_z", bufs=1, space="PSUM") as ps_z,
            tc.tile_pool(name="ps_misc", bufs=1, space="PSUM") as ps_misc,
        ):
            # ---------------- constants (all host-prepacked) ----------------
            id_f32 = const.tile([128, 128], f32)
            make_identity(nc, id_f32[:, :])
            id_bf = const.tile([128, 128], bf16)
            nc.vector.tensor_copy(id_bf[:, :], id_f32[:, :])

            # const loads spread across the three DMA queues so no single
            # queue delays the round-0 loads by the full preamble
            whidT = const.tile([128, NCH, 64], fp8)
            nc.sync.dma_start(out=whidT[:, :, :], in_=whidT_d)
            wentT = const.tile([128, 4 * ECH, A], bf16)
            nc.scalar.dma_start(out=wentT[:, :, :], in_=wentT_d)
            v128 = const.tile([128, 1], bf16)
            nc.scalar.dma_start(out=v128[:, :], in_=v_d)
            ltT = const.tile([128, ECH, T], bf16)
            nc.gpsimd.dma_start(out=ltT[:, :, :], in_=ltT_d)
            lt16 = const.tile([T, H2], bf16)
            nc.gpsimd.dma_start(out=lt16[:, :], in_=lt16_d)
            # preload the activation table while DMAs warm up (one table
            # serves Tanh and Exp; otherwise the 1.3us load lands in the
            # middle of the entity chain)
            atl = const.tile([1, 1], f32)
            nc.scalar.activation(atl[:, :], id_f32[0:1, 0:1], AF.Tanh)
            esall = const.tile([ROUND + 16, NR], f32)
            nc.gpsimd.memset(esall[:, :], 0.0)  # rows 16:32 of the last
            # column are dead (their halves land in alias rows 32:48)

            def entity_block(s, out):
                """Gather + latent-type + u2 for superbatch s (128 batches).
                Generator: yields between cross-engine stages so the driver
                can interleave them with round groups (keeps the serial
                chain out of PE's in-order queue).  Stores the u2 tile in
                out["u2sb"]."""
                srcT = []
                tiles = []
                for ent_d in (ent1_d, ent2_d):
                    ent = ent_pool.tile([SB, H2], bf16, tag="ent")
                    nc.gpsimd.dma_start(
                        out=ent[:, :], in_=ent_d[s * SB:(s + 1) * SB, :]
                    )
                    tiles.append(ent)
                yield
                # chunk 4 is 88 wide (600 = 4*128 + 88); no pad, no memset
                CW = [min(128, H2 - c * 128) for c in range(ECH)]
                for ent in tiles:
                    entT = ent_pool.tile([128, ECH, SB], bf16, tag="entT")
                    tp = ps_misc.tile([128, ECH, SB], bf16, tag="misc")
                    for c in range(ECH):
                        nc.tensor.transpose(
                            tp[0:CW[c], c, :],
                            ent[:, c * 128:c * 128 + CW[c]], id_bf[:, :]
                        )
                    nc.vector.tensor_copy(entT[:, 0:4, :], tp[:, 0:4, :])
                    nc.vector.tensor_copy(entT[0:CW[4], 4, :], tp[0:CW[4], 4, :])
                    yield
                    # latent-type logits [3, 128]
                    lg_ps = ps_misc.tile([T, SB], f32, tag="misc")
                    for c in range(ECH):
                        nc.tensor.matmul(
                            lg_ps[:, :], lhsT=ltT[0:CW[c], c, :],
                            rhs=entT[0:CW[c], c, :],
                            start=(c == 0), stop=(c == ECH - 1),
                        )
                    lgT_sb = ent_pool.tile([T, SB], f32, tag="lgT")
                    nc.vector.tensor_copy(lgT_sb[:, :], lg_ps[:, :])
                    yield
                    lg2_ps = ps_misc.tile([SB, T], f32, tag="misc")
                    nc.tensor.transpose(lg2_ps[:, :], lgT_sb[:, :], id_f32[0:T, 0:T])
                    expl = ent_pool.tile([SB, T], f32, tag="expl")
                    nc.scalar.activation(expl[:, :], lg2_ps[:, :], AF.Exp)
                    yield
                    ssum = ent_pool.tile([SB, 1], f32, tag="ssum")
                    nc.vector.reduce_sum(ssum[:, :], expl[:, :], axis=AX.X)
                    srec = ent_pool.tile([SB, 1], f32, tag="srec")
                    nc.vector.reciprocal(srec[:, :], ssum[:, :])
                    attw = ent_pool.tile([SB, T], f32, tag="attw")
                    nc.vector.tensor_scalar_mul(attw[:, :], expl[:, :], srec[:, 0:1])
                    yield
                    awT_ps = ps_misc.tile([T, SB], f32, tag="misc")
                    nc.tensor.transpose(awT_ps[:, :], attw[:, :], id_f32[:, :])
                    awT = ent_pool.tile([T, SB], bf16, tag="awT_sb")
                    nc.vector.tensor_copy(awT[:, :], awT_ps[:, :])
                    yield
                    # e_type = attw @ LT : [128, 600]
                    et = ent_pool.tile([SB, H2], bf16, tag="et_sb")
                    et_lo = ps_misc.tile([SB, 512], f32, tag="misc")
                    nc.tensor.matmul(
                        et_lo[:, :], lhsT=awT[:, :], rhs=lt16[:, 0:512],
                        start=True, stop=True,
                    )
                    nc.vector.tensor_copy(et[:, 0:512], et_lo[:, :])
                    yield
                    et_hi = ps_misc.tile([SB, 128], f32, tag="misc")
                    nc.tensor.matmul(
                        et_hi[:, 0:H2 - 512], lhsT=awT[:, :], rhs=lt16[:, 512:H2],
                        start=True, stop=True,
                    )
                    nc.vector.tensor_copy(et[:, 512:H2], et_hi[:, 0:H2 - 512])
                    yield
                    etT = ent_pool.tile([128, ECH, SB], bf16, tag="etT")
                    tp2 = ps_misc.tile([128, ECH, SB], bf16, tag="misc")
                    for c in range(ECH):
                        nc.tensor.transpose(
                            tp2[0:CW[c], c, :],
                            et[:, c * 128:c * 128 + CW[c]], id_bf[:, :]
                        )
                    nc.vector.tensor_copy(etT[:, 0:4, :], tp2[:, 0:4, :])
                    nc.vector.tensor_copy(etT[0:CW[4], 4, :], tp2[0:CW[4], 4, :])
                    yield
                    srcT.append((entT, etT))

                u2_ps = ps_misc.tile([A, SB], f32, tag="misc")
                order = [srcT[0][0], srcT[0][1], srcT[1][0], srcT[1][1]]
                k = 0
                for q in range(4):
                    for c in range(ECH):
                        cw = min(128, H2 - c * 128)
                        nc.tensor.matmul(
                            u2_ps[:, :],
                            lhsT=wentT[0:cw, q * ECH + c, :],
                            rhs=order[q][0:cw, c, :],
                            start=(k == 0), stop=(k == 19),
                        )
                        k += 1
                # padded to 64 rows (rows 50:64 zero) so the fused
                # relocate+u2-add and the id-matmul can cover the DR pad
                u2sb = ent_pool.tile([64, SB], bf16, tag="u2sb")
                nc.gpsimd.memset(u2sb[32:64, :], 0.0)
                nc.vector.tensor_copy(u2sb[0:A, :], u2_ps[:, :])
                out["u2sb"] = u2sb

            def load_hp(eng, ridx, hp, a, b):
                b0 = ridx * ROUND
                eng.dma_start(
                    out=hp[:, a:b, :],
                    in_=hid_d[:, b0 + a:b0 + b, :],
                )

            def emit_scores(sc_ps, pr, uT):
                for j in range(2 * GROUP):
                    half, jj = divmod(j, GROUP)
                    off = 64 * half
                    bl = pr * 2 * GROUP + j
                    nc.tensor.matmul(
                        sc_ps[:, bl:bl + 1],
                        lhsT=uT[off:off + A, jj * L:(jj + 1) * L],
                        rhs=v128[off:off + A, 0:1],
                        start=True, stop=True,
                    )

            carry = [None]  # (sc_ps, pair, uT) with scores not yet emitted

            def emit_groups(ridx, hp, hT, u2sb_fn, drain=None, nxt=None,
                            fin=None, last=False):
                """u1 + tanh for round ridx; group pairs share one PSUM bank
                (rows 0:64 / 64:128) so one tanh covers 8 batches.  Scores
                lag one pair, carried across rounds."""
                s, r = divmod(ridx, SB // ROUND)
                sc_ps = ps_sc.tile([L, ROUND], f32, tag="scT")
                for pr in range(NPAIR):
                    # group pair stacked on partitions (rows 0:64 / 64:128).
                    # The ISA requires matmul dst partition 0, so the odd
                    # group lands in a scratch bank and the otherwise-idle
                    # DVE relocates it — fusing in that group's u2 add for
                    # free; one tanh then covers 8 batches.
                    u1_ps = ps_u1.tile([128, GROUP * L], f32, tag="u1like")
                    hb_ps = ps_h.tile([64, GROUP * L], f32, tag="u1hi")
                    u2sb16 = u2sb_fn()
                    for half in range(2):
                        g = 2 * pr + half
                        dst = u1_ps if half == 0 else hb_ps
                        gsl = slice(g * GROUP * L, (g + 1) * GROUP * L)
                        for c in range(3):
                            nc.tensor.matmul(
                                dst[0:64, :],
                                lhsT=whidT[:, 2 * c:2 * c + 2, :],
                                rhs=hT[:, 2 * c:2 * c + 2, gsl],
                                start=(c == 0),
                                stop=(c == 2 and half == 1),
                                perf_mode=DR, skip_group_check=True,
                            )
                        b0r = r * ROUND + g * GROUP
                        u2r = u2sb16[0:64, b0r:b0r + GROUP]
                        u2b = bass.AP(
                            tensor=u2r.tensor, offset=u2r.offset,
                            ap=[u2r.ap[0], u2r.ap[1], [0, L]],
                        )
                        if half == 0:
                            # += u2 broadcast over tokens via identity matmul
                            nc.tensor.matmul(
                                u1_ps[0:64, :], lhsT=id_bf[0:64, 0:64],
                                rhs=u2b, start=False, stop=True,
                                skip_group_check=True,
                            )
                        else:
                            # relocate + u2 add in one DVE pass
                            nc.vector.scalar_tensor_tensor(
                                u1_ps[64:128, :].rearrange(
                                    "p (i l) -> p i l", i=GROUP),
                                hb_ps[:, :].rearrange(
                                    "p (i l) -> p i l", i=GROUP),
                                0.0, u2b,
                                op0=mybir.AluOpType.bypass,
                                op1=mybir.AluOpType.add,
                            )
                    uT = u_pool.tile([128, GROUP * L], bf16, tag="uT")
                    nc.scalar.activation(uT[:, :], u1_ps[:, :], AF.Tanh)
                    if drain is not None:
                        next(drain, None)
                        next(drain, None)
                    if nxt is not None:
                        # next rounds' Act/Pool loads, sliced between tanhs
                        # (hT prefetches two rounds ahead, hp one)
                        hp1, hT2, r1, r2 = nxt
                        p0, p1 = HP_SPLIT[0], HP_SPLIT[0] + HP_SPLIT[1]
                        mid = (p0 + p1) // 2
                        pm = (p1 + ROUND) // 2
                        if pr == 0:
                            if hT2 is not None:
                                load_act_ht(r2, hT2)
                        elif pr == 1:
                            if hp1 is not None:
                                load_hp(nc.scalar, r1, hp1, p0, mid)
                                load_hp(nc.gpsimd, r1, hp1, p1, pm)
                        elif pr == 2:
                            if hp1 is not None:
                                load_hp(nc.scalar, r1, hp1, mid, p1)
                                load_hp(nc.gpsimd, r1, hp1, pm, ROUND)
                    if carry[0] is not None:
                        emit_scores(*carry[0])
                        carry[0] = None
                    if last:
                        # no score lag in the last round: finish in halves
                        # as the scores become available to shorten the tail
                        emit_scores(sc_ps, pr, uT)
                        if pr == 1:
                            if fin is not None:
                                fin()
                            finish_round(ridx, hp, sc_ps, 0, ROUND // 2)
                        elif pr == NPAIR - 1:
                            finish_round(ridx, hp, sc_ps, ROUND // 2, ROUND)
                    else:
                        carry[0] = (sc_ps, pr, uT)
                        if pr == 1 and fin is not None:
                            fin()
                return hp, sc_ps

            zcur = [None, None]  # [zt_sb pair tile, zt_ps round tile]

            def finish_round(ridx, hp, sc_ps, q0=0, q1=ROUND):
                """Softmax numerator + z for batches q0:q1 of one round.
                Normally emitted (whole round) inside the NEXT round's
                group stream; the last round is finished in halves to
                shorten the serial tail.  z accumulates in PSUM across a
                round pair and ships one DMA per pair."""
                zslot = ridx % 2
                if q0 == 0:
                    if zslot == 0:
                        zsb_new = zs_pool.tile(
                            [128, ECH, 2, ROUND], f32, tag="zt_sb")
                        zcur[0] = zsb_new
                    zps_new = ps_z.tile([128, ECH, ROUND], f32, tag="zt")
                    zcur[1] = zps_new
                zt_sb, zt_ps = zcur
                n = q1 - q0
                scT_sb = small.tile([L, n], bf16, tag="scT_sb")
                nc.vector.tensor_copy(scT_sb[:, :], sc_ps[:, q0:q1])
                sc2_ps = ps_misc.tile([n, L], bf16, tag="misc")
                nc.tensor.transpose(sc2_ps[:, :], scT_sb[:, :], id_bf[:, :])
                exps = small.tile([n, L], bf16, tag="exps")
                nc.scalar.activation(exps[:, :], sc2_ps[:, :], AF.Exp)
                er0 = q0 if q0 % 32 == 0 else 32
                nc.vector.reduce_sum(esall[er0:er0 + n, ridx:ridx + 1],
                                     exps[:, :], axis=AX.X)
                aT_ps = ps_misc.tile([L, n], bf16, tag="misc")
                nc.tensor.transpose(aT_ps[:, :], exps[:, :], id_bf[0:n, 0:n])
                alphaT = small.tile([L, n], bf16, tag="alphaT")
                nc.vector.tensor_copy(alphaT[:, :], aT_ps[:, :])

                # zT[d, b] = sum_l hp[l, b, d] * exps[l, b]  (unnormalized)
                # chunk 4 covers features 472:600 (overlapping chunk 3) so
                # every PSUM row is written; the host drops the overlap
                for q in range(q0, q1):
                    for c in range(HCH):
                        oc = c * 128 if c < 4 else H2 - 128
                        nc.tensor.matmul(
                            zt_ps[:, c, q:q + 1],
                            lhsT=hp[:, q, oc:oc + 128],
                            rhs=alphaT[:, q - q0:q - q0 + 1],
                            start=True, stop=True,
                        )
                nc.vector.tensor_copy(
                    zt_sb[:, :, zslot, q0:q1], zt_ps[:, :, q0:q1])
                if zslot == 1 and q1 == ROUND:
                    nc.gpsimd.dma_start(
                        out=z_d[ridx // 2], in_=zt_sb[:, :, :, :]
                    )

            def ht_dma(eng, ridx, hT, ca, cb):
                eng.dma_start(
                    out=hT[:, ca:cb, :],
                    in_=ht8_d[ridx][:, ca * ROUND * L:cb * ROUND * L].rearrange(
                        "p (c n) -> p c n", c=cb - ca),
                )

            def load_ht_head(ridx, hT):
                """SP + Pool shares of round ridx's hT chunks."""
                c0, c1 = HT_SPLIT[0], HT_SPLIT[0] + HT_SPLIT[1]
                ht_dma(nc.sync, ridx, hT, 0, c0)
                ht_dma(nc.gpsimd, ridx, hT, c1, NCH)

            def load_act_ht(ridx, hT):
                c0, c1 = HT_SPLIT[0], HT_SPLIT[0] + HT_SPLIT[1]
                ht_dma(nc.scalar, ridx, hT, c0, c1)

            # ---------------- main schedule ----------------
            ent0, ent1 = {}, {}
            gen0 = entity_block(0, ent0)
            next(gen0)  # issue the gathers before anything else
            # hT is prefetched two rounds deep (it gates each round's start);
            # hp only one (first read a round and a half later, by z)
            hts = {}
            hps = {}
            for r0 in range(2):
                ht_t = ht_pool.tile([128, NCH, ROUND * L], fp8, tag="hT")
                hts[r0] = ht_t
                load_ht_head(r0, hts[r0])
                load_act_ht(r0, hts[r0])
            hp_t = hp_pool.tile([L, ROUND, H2], bf16, tag="hp")
            hps[0] = hp_t
            p0, p1 = HP_SPLIT[0], HP_SPLIT[0] + HP_SPLIT[1]
            load_hp(nc.sync, 0, hps[0], 0, p0)
            load_hp(nc.scalar, 0, hps[0], p0, p1)
            load_hp(nc.gpsimd, 0, hps[0], p1, ROUND)
            for _ in gen0:  # entity-0 chain runs under the round-0 loads
                pass
            gen1 = None
            pending = None
            for ridx in range(NR):
                r1, r2 = ridx + 1, ridx + 2
                if r2 < NR:
                    ht_t = ht_pool.tile([128, NCH, ROUND * L], fp8, tag="hT")
                    hts[r2] = ht_t
                    load_ht_head(r2, hts[r2])
                if r1 < NR:
                    hp_t = hp_pool.tile([L, ROUND, H2], bf16, tag="hp")
                    hps[r1] = hp_t
                    load_hp(nc.sync, r1, hps[r1], 0, HP_SPLIT[0])
                if ridx == 1:
                    gen1 = entity_block(1, ent1)
                if ridx == 4 and gen1 is not None:
                    for _ in gen1:
                        pass
                    gen1 = None
                ent = ent0 if ridx < 4 else ent1
                fin = None
                if pending is not None:
                    prv = pending
                    fin = lambda p=prv, r=ridx - 1: finish_round(r, *p)
                state = emit_groups(
                    ridx, hps[ridx], hts[ridx], lambda e=ent: e["u2sb"],
                    drain=gen1,
                    nxt=(hps.get(r1), hts.get(r2), r1, r2),
                    fin=fin, last=(LAST_HALVES and ridx == NR - 1),
                )
                pending = state
            if not LAST_HALVES:
                emit_scores(*carry[0])
                carry[0] = None
                finish_round(NR - 1, *pending)
            # SP is idle at the end; es ships there, parallel to the z store
            nc.sync.dma_start(out=es_d, in_=esall[:, :])

    nc.compile()
    return nc


def _get_nc():
    if "nc" not in _CACHE:
        _CACHE["nc"] = _build_bass()
    return _CACHE["nc"]


def _to_bf16(x):
    import ml_dtypes
    return np.asarray(x, dtype=np.float32).astype(ml_dtypes.bfloat16)


def _to_fp8(x):
    import ml_dtypes
    return np.asarray(x, dtype=np.float32).astype(ml_dtypes.float8_e4m3)


def _prep_weights(inputs):
    """Host-side weight transposition/padding into the chunk layouts."""
    w_hid = np.asarray(inputs["W_hid"], dtype=np.float32)   # [50, 700]
    w_ent = np.asarray(inputs["W_ent"], dtype=np.float32)   # [50, 2400]
    lt = np.asarray(inputs["latent_types"], dtype=np.float32)  # [3, 600]
    v = np.asarray(inputs["v"], dtype=np.float32)           # [50, 1]

    # whidT [128, 6, 64]: chunks 0-4 = hidden features, chunk 5 = pos;
    # output columns padded 50 -> 64 (DoubleRow needs M in {64, 128})
    whidT = np.zeros((128, NCH, 64), np.float32)
    wf = w_hid.T  # [700, 50]
    for c in range(HCH):
        cw = min(128, H2 - c * 128)
        whidT[0:cw, c, 0:A] = wf[c * 128:c * 128 + cw]
    whidT[0:POSF, 5, 0:A] = wf[H2:H2 + POSF]

    # wentT [128, 20, 50]: quarter q (e1, e1t, e2, e2t), chunk c of 640-pad
    wentT = np.zeros((128, 4 * ECH, A), np.float32)
    we = w_ent.T  # [2400, 50]
    for q in range(4):
        for c in range(ECH):
            lo = q * H2 + c * 128
            cw = min(128, (q + 1) * H2 - lo)
            if cw > 0:
                wentT[0:cw, q * ECH + c, :] = we[lo:lo + cw]

    # ltT [128, 5, 3] transposed latent type chunks
    ltT = np.zeros((128, ECH, T), np.float32)
    ltf = lt.T  # [600, 3]
    for c in range(ECH):
        cw = min(128, H2 - c * 128)
        ltT[0:cw, c, :] = ltf[c * 128:c * 128 + cw]

    # v replicated at partition offsets 0 and 64 (paired-group scores)
    v128 = np.zeros((128, 1), np.float32)
    v128[0:A] = v
    v128[64:64 + A] = v

    return {
        "whidT": _to_fp8(whidT),
        "wentT": _to_bf16(wentT),
        "ltT": _to_bf16(ltT),
        "lt16": _to_bf16(lt),
        "v128": _to_bf16(v128),
    }


def make_in_maps(inputs):
    import ml_dtypes
    hidden16 = _to_bf16(inputs["hidden"])                    # [B, L, 600]
    hid_f = np.asarray(inputs["hidden"], np.float32)
    # ht8 [128, 6, B, L]: feature-major fp8 hidden chunks + pos chunk 5
    ht8 = np.zeros((128, NCH, B, L), ml_dtypes.float8_e4m3)
    hfT = hid_f.transpose(2, 0, 1)                           # [600, B, L]
    for c in range(HCH):
        cw = min(128, H2 - c * 128)
        ht8[0:cw, c] = hfT[c * 128:c * 128 + cw].astype(ml_dtypes.float8_e4m3)
    pos = np.concatenate(
        [np.asarray(inputs["pos1_emb"], np.float32),
         np.asarray(inputs["pos2_emb"], np.float32)], axis=2
    )                                                        # [B, L, 100]
    ht8[0:POSF, 5] = pos.transpose(2, 0, 1).astype(ml_dtypes.float8_e4m3)

    e1 = np.asarray(inputs["entity1_idx"]).astype(np.int64)
    e2 = np.asarray(inputs["entity2_idx"]).astype(np.int64)
    weights = _prep_weights(inputs)

    bix = np.arange(B)
    ent1 = hidden16[bix, e1]                                 # [B, H2] bf16
    ent2 = hidden16[bix, e2]
    in_maps = []
    for c in range(NCORES):
        sl = slice(c * BC, (c + 1) * BC)
        # hid_l: l-major [L, BC, H2]
        hid_l = np.ascontiguousarray(hidden16[sl].transpose(1, 0, 2))
        # ht8r: round-major [NR, 128, NCH * ROUND * L]
        ht8r = np.ascontiguousarray(
            ht8[:, :, sl, :].reshape(128, NCH, NR, ROUND * L)
            .transpose(2, 0, 1, 3)).reshape(NR, 128, NCH * ROUND * L)
        in_maps.append({
            "hid_l": hid_l,
            "ht8r": ht8r,
            "ent1": np.ascontiguousarray(ent1[sl]),
            "ent2": np.ascontiguousarray(ent2[sl]),
            **weights,
        })
    return in_maps


def unshard_z(zt, es):
    # zt: [NR//2, 128, ECH, 2, ROUND] with
    #   z[(2*pair + s)*ROUND + q, c*128 + p] = zt[pair, p, c, s, q]
    # except chunk 4 holds features 472:600 (overlaps chunk 3)
    z = np.transpose(np.asarray(zt, dtype=np.float32), (0, 3, 4, 2, 1))
    z = z.reshape(BC, ECH * 128)
    z = np.concatenate([z[:, 0:512], z[:, 512 + 40:640]], axis=1)
    # es: [ROUND+16, NR]; batch r*ROUND+q -> es[q, r], except the last
    # round's second half which lands in alias rows 32:48 when the last
    # round is finished in halves
    es = np.asarray(es, dtype=np.float32).copy()
    if LAST_HALVES:
        es[16:ROUND, NR - 1] = es[ROUND:ROUND + 16, NR - 1]
    den = es[:ROUND].T.reshape(BC, 1)
    return z / den


def kernel(**inputs):
    from concourse.bass_utils import run_bass_kernel_spmd

    nc = _get_nc()
    in_maps = make_in_maps(inputs)
    res = None
    for attempt in range(3):
        try:
            res = run_bass_kernel_spmd(
                nc, in_maps, core_ids=list(range(NCORES)))
            break
        except Exception:
            # the axon transport occasionally drops a run; retry
            if attempt == 2:
                raise
    _CACHE["last_res"] = res
    outs = [unshard_z(r["z"], r["esum"]) for r in res.results]
    return np.concatenate(outs, axis=0).astype(np.float32)


# revision 105
# speedup vs baseline: 1.1329x; 1.0451x over previous
"""EntityAwareAttention Trainium2 kernel, v3.

Per batch b of B=2048:
    hid_e{1,2} = hidden[b, e{1,2}_idx[b]]                       # [600]
    e{1,2}_type = softmax(hid_e @ LT.T) @ LT                    # [600], T=3
    u1 = concat(hidden, pos1, pos2) @ W_hid.T                   # [128, 50]
    u2 = concat(hid_e1, e1_type, hid_e2, e2_type) @ W_ent.T     # [50]
    u = tanh(u1 + u2); scores = u @ v; alpha = softmax(scores)  # [128]
    z = alpha @ hidden[b]                                       # [600]

Pure data parallel over batch: 8 cores x 256 batches, weights replicated.

v3 design (~2x the v2 kernel on the CoreSim cost model):
  - v2's bottleneck was PSUM evacuation of on-chip PE transposes
    (DVE 87% / Act 85% busy, nearly all tensor-copy).  v3 deletes the
    transpose pipeline entirely: the host pre-packs hidden a second
    time in feature-major fp8 (ht8 [128, 6ch, BC*L], pos folded in as
    chunk 5), DMA'd straight into the u1 rhs slot.  Token-major bf16
    hidden is still loaded for the z path (z matmuls have free-size-1
    outputs, which the PE does at negligible cost).
  - u1 matmuls in fp8 DoubleRow, group pairs stacked vertically in one
    PSUM bank (rows 0:64 / 64:128) so a single tanh covers 8 batches;
    v is host-replicated to partitions 64-113 so the per-batch score
    matmuls can read either half.
  - Scores lag one group pair (carried across rounds) so the PE never
    waits on tanh; softmax is unnormalized (host divides); z goes
    PSUM -> DRAM directly, batched 2 rounds per DMA; esum accumulates
    on-chip all 8 rounds and ships once.
  - DMA is the cost floor (hidden 1.5 copies + pos ~ 24.3us/round of
    queue time) and only SP/Act/Pool can issue DMAs, so loads are
    split SP: 3 hT chunks + 9 hp batches, Act: 1 + 11 (Act also runs
    tanh/exp), Pool: 2 + 12 (+ gathers and stores).  Entity/u2 chain
    unchanged from v2 except PSUM evacs moved Act -> DVE (DVE is
    otherwise idle; Act is a DMA queue now).
"""

import numpy as np

B, L, H2, PP, A, T = 2048, 128, 600, 50, 50, 3
NCORES = 8
BC = B // NCORES   # 256 batches per core
SB = 128           # superbatch for the entity/u2 pipeline
ROUND = 32         # batches per round
GROUP = 4          # batches per u1 matmul group (N = 4*128 = 512)
NPAIR = ROUND // (2 * GROUP)  # group pairs per round
NR = BC // ROUND   # rounds per core
NCH = 6            # rhs feature chunks (5 hidden + 1 pos)
HCH = 5            # hidden chunks (4x128 + 88)
EPAD = 640         # entity vectors padded to 5x128
ECH = 5
POSF = 2 * PP      # 100 pos features

# DMA queue split for the per-round loads (SP / Act / Pool)
HT_SPLIT = (3, 1, 2)    # of the 6 ht8 chunks
HP_SPLIT = (10, 9, 13)  # of the 32 hp batches
HP_SPLIT_ALT = None     # if set, used on odd rounds
ZT_QUEUE = "sp"         # queue for the z pair stores: pool/act/sp
LAST_HALVES = True      # finish the last round in halves (shorter tail)

_CACHE = {}


def _build_bass():
    import concourse.bass as bass
    import concourse.bacc as bacc
    import concourse.tile as tile
    from concourse import mybir
    from concourse.masks import make_identity

    f32 = mybir.dt.float32
    bf16 = mybir.dt.bfloat16
    fp8 = mybir.dt.float8e4
    i32 = mybir.dt.int32
    AF = mybir.ActivationFunctionType
    AX = mybir.AxisListType
    DR = mybir.MatmulPerfMode.DoubleRow

    nc = bacc.Bacc("TRN2", debug=False, target_bir_lowering=False)

    # hidden is staged l-major ([L, BC, H2]) and ht8 round-major so each
    # round's load is one long contiguous run per partition
    hid_d = nc.dram_tensor("hid_l", [L, BC, H2], bf16, kind="ExternalInput").ap()
    ht8_d = nc.dram_tensor(
        "ht8r", [NR, 128, NCH * ROUND * L], fp8, kind="ExternalInput").ap()
    # entity rows host-gathered (pure indexing, like the index math the
    # host already does): [BC, H2] bf16 each
    ent1_d = nc.dram_tensor("ent1", [BC, H2], bf16, kind="ExternalInput").ap()
    ent2_d = nc.dram_tensor("ent2", [BC, H2], bf16, kind="ExternalInput").ap()
    # host-pretransposed weights
    whidT_d = nc.dram_tensor("whidT", [128, NCH, 64], fp8, kind="ExternalInput").ap()
    wentT_d = nc.dram_tensor("wentT", [128, 4 * ECH, A], bf16, kind="ExternalInput").ap()
    ltT_d = nc.dram_tensor("ltT", [128, ECH, T], bf16, kind="ExternalInput").ap()
    lt16_d = nc.dram_tensor("lt16", [T, H2], bf16, kind="ExternalInput").ap()
    v_d = nc.dram_tensor("v128", [128, 1], bf16, kind="ExternalInput").ap()
    z_d = nc.dram_tensor(
        "z", [NR // 2, 128, ECH, 2, ROUND], f32, kind="ExternalOutput"
    ).ap()
    # per-batch softmax denominators; z is stored unnormalized and the
    # host divides (keeps the recip/scale off the round critical path)
    # rows 32:48 alias the last round's second half (a partition-start-16
    # write is not ISA-legal, so it lands at partition 32 instead)
    es_d = nc.dram_tensor(
        "esum", [ROUND + 16, NR], f32, kind="ExternalOutput").ap()

    with tile.TileContext(nc) as tc, nc.allow_non_contiguous_dma(
        "partition-dim-last APs for the parallel DMA fabric"
    ):
        with (
            tc.tile_pool(name="const", bufs=1) as const,
            tc.tile_pool(name="hp_pool", bufs=3) as hp_pool,
            tc.tile_pool(name="ht_pool", bufs=3) as ht_pool,
            tc.tile_pool(name="u_pool", bufs=2) as u_pool,
            tc.tile_pool(name="ent_pool", bufs=2) as ent_pool,
            tc.tile_pool(name="small", bufs=4) as small,
            tc.tile_pool(name="zs_pool", bufs=2) as zs_pool,
            tc.tile_pool(name="ps_u1", bufs=2, space="PSUM") as ps_u1,
            tc.tile_pool(name="ps_h", bufs=2, space="PSUM") as ps_h,
            tc.tile_pool(name="ps_sc", bufs=2, space="PSUM") as ps_sc,
            tc.tile_pool(name="ps_z", bufs=1, space="PSUM") as ps_z,
            tc.tile_pool(name="ps_misc", bufs=1, space="PSUM") as ps_misc,
        ):
            # ---------------- constants (all host-prepacked) ----------------
            id_f32 = const.tile([128, 128], f32)
            make_identity(nc, id_f32[:, :])
            id_bf = const.tile([128, 128], bf16)
            nc.vector.tensor_copy(id_bf[:, :], id_f32[:, :])

            # const loads spread across the three DMA queues so no single
            # queue delays the round-0 loads by the full preamble
            whidT = const.tile([128, NCH, 64], fp8)
            nc.sync.dma_start(out=whidT[:, :, :], in_=whidT_d)
            wentT = const.tile([128, 4 * ECH, A], bf16)
            nc.scalar.dma_start(out=wentT[:, :, :], in_=wentT_d)
            v128 = const.tile([128, 1], bf16)
            nc.scalar.dma_start(out=v128[:, :], in_=v_d)
            ltT = const.tile([128, ECH, T], bf16)
            nc.gpsimd.dma_start(out=ltT[:, :, :], in_=ltT_d)
            lt16 = const.tile([T, H2], bf16)
            nc.gpsimd.dma_start(out=lt16[:, :], in_=lt16_d)
            # preload the activation table while DMAs warm up (one table
            # serves Tanh and Exp; otherwise the 1.3us load lands in the
            # middle of the entity chain)
            atl = const.tile([1, 1], f32)
            nc.scalar.activation(atl[:, :], id_f32[0:1, 0:1], AF.Tanh)
            esall = const.tile([ROUND + 16, NR], f32)
            nc.gpsimd.memset(esall[:, :], 0.0)  # rows 16:32 of the last
            # column are dead (their halves land in alias rows 32:48)

            def entity_block(s, out):
                """Gather + latent-type + u2 for superbatch s (128 batches).
                Generator: yields between cross-engine stages so the driver
                can interleave them with round groups (keeps the serial
                chain out of PE's in-order queue).  Stores the u2 tile in
                out["u2sb"]."""
                srcT = []
                tiles = []
                for ent_d in (ent1_d, ent2_d):
                    ent = ent_pool.tile([SB, H2], bf16, tag="ent")
                    nc.gpsimd.dma_start(
                        out=ent[:, :], in_=ent_d[s * SB:(s + 1) * SB, :]
                    )
                    tiles.append(ent)
                yield
                # chunk 4 is 88 wide (600 = 4*128 + 88); no pad, no memset
                CW = [min(128, H2 - c * 128) for c in range(ECH)]
                for ent in tiles:
                    entT = ent_pool.tile([128, ECH, SB], bf16, tag="entT")
                    tp = ps_misc.tile([128, ECH, SB], bf16, tag="misc")
                    for c in range(ECH):
                        nc.tensor.transpose(
                            tp[0:CW[c], c, :],
                            ent[:, c * 128:c * 128 + CW[c]], id_bf[:, :]
                        )
                    nc.vector.tensor_copy(entT[:, 0:4, :], tp[:, 0:4, :])
                    nc.vector.tensor_copy(entT[0:CW[4], 4, :], tp[0:CW[4], 4, :])
                    yield
                    # latent-type logits [3, 128]
                    lg_ps = ps_misc.tile([T, SB], f32, tag="misc")
                    for c in range(ECH):
                        nc.tensor.matmul(
                            lg_ps[:, :], lhsT=ltT[0:CW[c], c, :],
                            rhs=entT[0:CW[c], c, :],
                            start=(c == 0), stop=(c == ECH - 1),
                        )
                    lgT_sb = ent_pool.tile([T, SB], f32, tag="lgT")
                    nc.vector.tensor_copy(lgT_sb[:, :], lg_ps[:, :])
                    yield
                    lg2_ps = ps_misc.tile([SB, T], f32, tag="misc")
                    nc.tensor.transpose(lg2_ps[:, :], lgT_sb[:, :], id_f32[0:T, 0:T])
                    expl = ent_pool.tile([SB, T], f32, tag="expl")
                    nc.scalar.activation(expl[:, :], lg2_ps[:, :], AF.Exp)
                    yield
                    ssum = ent_pool.tile([SB, 1], f32, tag="ssum")
                    nc.vector.reduce_sum(ssum[:, :], expl[:, :], axis=AX.X)
                    srec = ent_pool.tile([SB, 1], f32, tag="srec")
                    nc.vector.reciprocal(srec[:, :], ssum[:, :])
                    attw = ent_pool.tile([SB, T], f32, tag="attw")
                    nc.vector.tensor_scalar_mul(attw[:, :], expl[:, :], srec[:, 0:1])
                    yield
                    awT_ps = ps_misc.tile([T, SB], f32, tag="misc")
                    nc.tensor.transpose(awT_ps[:, :], attw[:, :], id_f32[:, :])
                    awT = ent_pool.tile([T, SB], bf16, tag="awT_sb")
                    nc.vector.tensor_copy(awT[:, :], awT_ps[:, :])
                    yield
                    # e_type = attw @ LT : [128, 600]
                    et = ent_pool.tile([SB, H2], bf16, tag="et_sb")
                    et_lo = ps_misc.tile([SB, 512], f32, tag="misc")
                    nc.tensor.matmul(
                        et_lo[:, :], lhsT=awT[:, :], rhs=lt16[:, 0:512],
                        start=True, stop=True,
                    )
                    nc.vector.tensor_copy(et[:, 0:512], et_lo[:, :])
                    yield
                    et_hi = ps_misc.tile([SB, 128], f32, tag="misc")
                    nc.tensor.matmul(
                        et_hi[:, 0:H2 - 512], lhsT=awT[:, :], rhs=lt16[:, 512:H2],
                        start=True, stop=True,
                    )
                    nc.vector.tensor_copy(et[:, 512:H2], et_hi[:, 0:H2 - 512])
                    yield
                    etT = ent_pool.tile([128, ECH, SB], bf16, tag="etT")
                    tp2 = ps_misc.tile([128, ECH, SB], bf16, tag="misc")
                    for c in range(ECH):
                        nc.tensor.transpose(
                            tp2[0:CW[c], c, :],
                            et[:, c * 128:c * 128 + CW[c]], id_bf[:, :]
                        )
                    nc.vector.tensor_copy(etT[:, 0:4, :], tp2[:, 0:4, :])
                    nc.vector.tensor_copy(etT[0:CW[4], 4, :], tp2[0:CW[4], 4, :])
                    yield
                    srcT.append((entT, etT))

                u2_ps = ps_misc.tile([A, SB], f32, tag="misc")
                order = [srcT[0][0], srcT[0][1], srcT[1][0], srcT[1][1]]
                k = 0
                for q in range(4):
                    for c in range(ECH):
                        cw = min(128, H2 - c * 128)
                        nc.tensor.matmul(
                            u2_ps[:, :],
                            lhsT=wentT[0:cw, q * ECH + c, :],
                            rhs=order[q][0:cw, c, :],
                            start=(k == 0), stop=(k == 19),
                        )
                        k += 1
                # padded to 64 rows (rows 50:64 zero) so the fused
                # relocate+u2-add and the id-matmul can cover the DR pad
                u2sb = ent_pool.tile([64, SB], bf16, tag="u2sb")
                nc.gpsimd.memset(u2sb[32:64, :], 0.0)
                nc.vector.tensor_copy(u2sb[0:A, :], u2_ps[:, :])
                out["u2sb"] = u2sb

            def hp_split(r):
                if HP_SPLIT_ALT is not None and r % 2 == 1:
                    return HP_SPLIT_ALT
                return HP_SPLIT

            ZT_ENG = {"pool": nc.gpsimd, "act": nc.scalar,
                      "sp": nc.sync}[ZT_QUEUE]

            def load_hp(eng, ridx, hp, a, b):
                b0 = ridx * ROUND
                eng.dma_start(
                    out=hp[:, a:b, :],
                    in_=hid_d[:, b0 + a:b0 + b, :],
                )

            def emit_scores(sc_ps, pr, uT):
                for j in range(2 * GROUP):
                    half, jj = divmod(j, GROUP)
                    off = 64 * half
                    bl = pr * 2 * GROUP + j
                    nc.tensor.matmul(
                        sc_ps[:, bl:bl + 1],
                        lhsT=uT[off:off + A, jj * L:(jj + 1) * L],
                        rhs=v128[off:off + A, 0:1],
                        start=True, stop=True,
                    )

            carry = [None]  # (sc_ps, pair, uT) with scores not yet emitted

            def emit_groups(ridx, hp, hT, u2sb_fn, drain=None, nxt=None,
                            fin=None, last=False):
                """u1 + tanh for round ridx; group pairs share one PSUM bank
                (rows 0:64 / 64:128) so one tanh covers 8 batches.  Scores
                lag one pair, carried across rounds."""
                s, r = divmod(ridx, SB // ROUND)
                sc_ps = ps_sc.tile([L, ROUND], f32, tag="scT")
                for pr in range(NPAIR):
                    # group pair stacked on partitions (rows 0:64 / 64:128).
                    # The ISA requires matmul dst partition 0, so the odd
                    # group lands in a scratch bank and the otherwise-idle
                    # DVE relocates it — fusing in that group's u2 add for
                    # free; one tanh then covers 8 batches.
                    u1_ps = ps_u1.tile([128, GROUP * L], f32, tag="u1like")
                    hb_ps = ps_h.tile([64, GROUP * L], f32, tag="u1hi")
                    u2sb16 = u2sb_fn()
                    for half in range(2):
                        g = 2 * pr + half
                        dst = u1_ps if half == 0 else hb_ps
                        gsl = slice(g * GROUP * L, (g + 1) * GROUP * L)
                        for c in range(3):
                            nc.tensor.matmul(
                                dst[0:64, :],
                                lhsT=whidT[:, 2 * c:2 * c + 2, :],
                                rhs=hT[:, 2 * c:2 * c + 2, gsl],
                                start=(c == 0),
                                stop=(c == 2 and half == 1),
                                perf_mode=DR, skip_group_check=True,
                            )
                        b0r = r * ROUND + g * GROUP
                        u2r = u2sb16[0:64, b0r:b0r + GROUP]
                        u2b = bass.AP(
                            tensor=u2r.tensor, offset=u2r.offset,
                            ap=[u2r.ap[0], u2r.ap[1], [0, L]],
                        )
                        if half == 0:
                            # += u2 broadcast over tokens via identity matmul
                            nc.tensor.matmul(
                                u1_ps[0:64, :], lhsT=id_bf[0:64, 0:64],
                                rhs=u2b, start=False, stop=True,
                                skip_group_check=True,
                            )
                        else:
                            # relocate + u2 add in one DVE pass
                            nc.vector.scalar_tensor_tensor(
                                u1_ps[64:128, :].rearrange(
                                    "p (i l) -> p i l", i=GROUP),
                                hb_ps[:, :].rearrange(
                                    "p (i l) -> p i l", i=GROUP),
                                0.0, u2b,
                                op0=mybir.AluOpType.bypass,
                                op1=mybir.AluOpType.add,
                            )
                    uT = u_pool.tile([128, GROUP * L], bf16, tag="uT")
                    nc.scalar.activation(uT[:, :], u1_ps[:, :], AF.Tanh)
                    if drain is not None:
                        next(drain, None)
                        next(drain, None)
                    if nxt is not None:
                        # next rounds' Act/Pool loads, sliced between tanhs
                        # (hT prefetches two rounds ahead, hp one)
                        hp1, hT2, r1, r2 = nxt
                        spl = hp_split(r1)
                        p0, p1 = spl[0], spl[0] + spl[1]
                        mid = (p0 + p1) // 2
                        pm = (p1 + ROUND) // 2
                        if pr == 0:
                            if hT2 is not None:
                                load_act_ht(r2, hT2)
                        elif pr == 1:
                            if hp1 is not None:
                                load_hp(nc.scalar, r1, hp1, p0, mid)
                                load_hp(nc.gpsimd, r1, hp1, p1, pm)
                        elif pr == 2:
                            if hp1 is not None:
                                load_hp(nc.scalar, r1, hp1, mid, p1)
                                load_hp(nc.gpsimd, r1, hp1, pm, ROUND)
                    if carry[0] is not None:
                        emit_scores(*carry[0])
                        carry[0] = None
                    if last:
                        # no score lag in the last round: finish in halves
                        # as the scores become available to shorten the tail
                        emit_scores(sc_ps, pr, uT)
                        if pr == 1:
                            if fin is not None:
                                fin()
                            finish_round(ridx, hp, sc_ps, 0, ROUND // 2)
                        elif pr == NPAIR - 1:
                            finish_round(ridx, hp, sc_ps, ROUND // 2, ROUND)
                    else:
                        carry[0] = (sc_ps, pr, uT)
                        if pr == 1 and fin is not None:
                            fin()
                return hp, sc_ps

            zcur = [None, None]  # [zt_sb pair tile, zt_ps round tile]

            def finish_round(ridx, hp, sc_ps, q0=0, q1=ROUND):
                """Softmax numerator + z for batches q0:q1 of one round.
                Normally emitted (whole round) inside the NEXT round's
                group stream; the last round is finished in halves to
                shorten the serial tail.  z accumulates in PSUM across a
                round pair and ships one DMA per pair."""
                zslot = ridx % 2
                if q0 == 0:
                    if zslot == 0:
                        zsb_new = zs_pool.tile(
                            [128, ECH, 2, ROUND], f32, tag="zt_sb")
                        zcur[0] = zsb_new
                    zps_new = ps_z.tile([128, ECH, ROUND], f32, tag="zt")
                    zcur[1] = zps_new
                zt_sb, zt_ps = zcur
                n = q1 - q0
                scT_sb = small.tile([L, n], bf16, tag="scT_sb")
                nc.vector.tensor_copy(scT_sb[:, :], sc_ps[:, q0:q1])
                sc2_ps = ps_misc.tile([n, L], bf16, tag="misc")
                nc.tensor.transpose(sc2_ps[:, :], scT_sb[:, :], id_bf[:, :])
                exps = small.tile([n, L], bf16, tag="exps")
                nc.scalar.activation(exps[:, :], sc2_ps[:, :], AF.Exp)
                er0 = q0 if q0 % 32 == 0 else 32
                nc.vector.reduce_sum(esall[er0:er0 + n, ridx:ridx + 1],
                                     exps[:, :], axis=AX.X)
                aT_ps = ps_misc.tile([L, n], bf16, tag="misc")
                nc.tensor.transpose(aT_ps[:, :], exps[:, :], id_bf[0:n, 0:n])
                alphaT = small.tile([L, n], bf16, tag="alphaT")
                nc.vector.tensor_copy(alphaT[:, :], aT_ps[:, :])

                # zT[d, b] = sum_l hp[l, b, d] * exps[l, b]  (unnormalized)
                # chunk 4 covers features 472:600 (overlapping chunk 3) so
                # every PSUM row is written; the host drops the overlap
                for q in range(q0, q1):
                    for c in range(HCH):
                        oc = c * 128 if c < 4 else H2 - 128
                        nc.tensor.matmul(
                            zt_ps[:, c, q:q + 1],
                            lhsT=hp[:, q, oc:oc + 128],
                            rhs=alphaT[:, q - q0:q - q0 + 1],
                            start=True, stop=True,
                        )
                nc.vector.tensor_copy(
                    zt_sb[:, :, zslot, q0:q1], zt_ps[:, :, q0:q1])
                if zslot == 1 and q1 == ROUND:
                    ZT_ENG.dma_start(
                        out=z_d[ridx // 2], in_=zt_sb[:, :, :, :]
                    )

            def ht_dma(eng, ridx, hT, ca, cb):
                eng.dma_start(
                    out=hT[:, ca:cb, :],
                    in_=ht8_d[ridx][:, ca * ROUND * L:cb * ROUND * L].rearrange(
                        "p (c n) -> p c n", c=cb - ca),
                )

            def load_ht_head(ridx, hT):
                """SP + Pool shares of round ridx's hT chunks."""
                c0, c1 = HT_SPLIT[0], HT_SPLIT[0] + HT_SPLIT[1]
                ht_dma(nc.sync, ridx, hT, 0, c0)
                ht_dma(nc.gpsimd, ridx, hT, c1, NCH)

            def load_act_ht(ridx, hT):
                c0, c1 = HT_SPLIT[0], HT_SPLIT[0] + HT_SPLIT[1]
                ht_dma(nc.scalar, ridx, hT, c0, c1)

            # ---------------- main schedule ----------------
            ent0, ent1 = {}, {}
            gen0 = entity_block(0, ent0)
            next(gen0)  # issue the gathers before anything else
            # hT is prefetched two rounds deep (it gates each round's start);
            # hp only one (first read a round and a half later, by z)
            hts = {}
            hps = {}
            for r0 in range(2):
                ht_t = ht_pool.tile([128, NCH, ROUND * L], fp8, tag="hT")
                hts[r0] = ht_t
                load_ht_head(r0, hts[r0])
                load_act_ht(r0, hts[r0])
            hp_t = hp_pool.tile([L, ROUND, H2], bf16, tag="hp")
            hps[0] = hp_t
            spl0 = hp_split(0)
            p0, p1 = spl0[0], spl0[0] + spl0[1]
            load_hp(nc.sync, 0, hps[0], 0, p0)
            load_hp(nc.scalar, 0, hps[0], p0, p1)
            load_hp(nc.gpsimd, 0, hps[0], p1, ROUND)
            for _ in gen0:  # entity-0 chain runs under the round-0 loads
                pass
            gen1 = None
            pending = None
            for ridx in range(NR):
                r1, r2 = ridx + 1, ridx + 2
                if r2 < NR:
                    ht_t = ht_pool.tile([128, NCH, ROUND * L], fp8, tag="hT")
                    hts[r2] = ht_t
                    load_ht_head(r2, hts[r2])
                if r1 < NR:
                    hp_t = hp_pool.tile([L, ROUND, H2], bf16, tag="hp")
                    hps[r1] = hp_t
                    load_hp(nc.sync, r1, hps[r1], 0, hp_split(r1)[0])
                if ridx == 1:
                    gen1 = entity_block(1, ent1)
                if ridx == 4 and gen1 is not None:
                    for _ in gen1:
                        pass
                    gen1 = None
                ent = ent0 if ridx < 4 else ent1
                fin = None
                if pending is not None:
                    prv = pending
                    fin = lambda p=prv, r=ridx - 1: finish_round(r, *p)
                state = emit_groups(
                    ridx, hps[ridx], hts[ridx], lambda e=ent: e["u2sb"],
                    drain=gen1,
                    nxt=(hps.get(r1), hts.get(r2), r1, r2),
                    fin=fin, last=(LAST_HALVES and ridx == NR - 1),
                )
                pending = state
            if not LAST_HALVES:
                emit_scores(*carry[0])
                carry[0] = None
                finish_round(NR - 1, *pending)
            # SP is idle at the end; es ships there, parallel to the z store
            nc.sync.dma_start(out=es_d, in_=esall[:, :])

    nc.compile()
    return nc


def _get_nc():
    if "nc" not in _CACHE:
        _CACHE["nc"] = _build_bass()
    return _CACHE["nc"]


def _to_bf16(x):
    import ml_dtypes
    return np.asarray(x, dtype=np.float32).astype(ml_dtypes.bfloat16)


def _to_fp8(x):
    import ml_dtypes
    return np.asarray(x, dtype=np.float32).astype(ml_dtypes.float8_e4m3)


def _prep_weights(inputs):
    """Host-side weight transposition/padding into the chunk layouts."""
    w_hid = np.asarray(inputs["W_hid"], dtype=np.float32)   # [50, 700]
    w_ent = np.asarray(inputs["W_ent"], dtype=np.float32)   # [50, 2400]
    lt = np.asarray(inputs["latent_types"], dtype=np.float32)  # [3, 600]
    v = np.asarray(inputs["v"], dtype=np.float32)           # [50, 1]

    # whidT [128, 6, 64]: chunks 0-4 = hidden features, chunk 5 = pos;
    # output columns padded 50 -> 64 (DoubleRow needs M in {64, 128})
    whidT = np.zeros((128, NCH, 64), np.float32)
    wf = w_hid.T  # [700, 50]
    for c in range(HCH):
        cw = min(128, H2 - c * 128)
        whidT[0:cw, c, 0:A] = wf[c * 128:c * 128 + cw]
    whidT[0:POSF, 5, 0:A] = wf[H2:H2 + POSF]

    # wentT [128, 20, 50]: quarter q (e1, e1t, e2, e2t), chunk c of 640-pad
    wentT = np.zeros((128, 4 * ECH, A), np.float32)
    we = w_ent.T  # [2400, 50]
    for q in range(4):
        for c in range(ECH):
            lo = q * H2 + c * 128
            cw = min(128, (q + 1) * H2 - lo)
            if cw > 0:
                wentT[0:cw, q * ECH + c, :] = we[lo:lo + cw]

    # ltT [128, 5, 3] transposed latent type chunks
    ltT = np.zeros((128, ECH, T), np.float32)
    ltf = lt.T  # [600, 3]
    for c in range(ECH):
        cw = min(128, H2 - c * 128)
        ltT[0:cw, c, :] = ltf[c * 128:c * 128 + cw]

    # v replicated at partition offsets 0 and 64 (paired-group scores)
    v128 = np.zeros((128, 1), np.float32)
    v128[0:A] = v
    v128[64:64 + A] = v

    return {
        "whidT": _to_fp8(whidT),
        "wentT": _to_bf16(wentT),
        "ltT": _to_bf16(ltT),
        "lt16": _to_bf16(lt),
        "v128": _to_bf16(v128),
    }


def make_in_maps(inputs):
    import ml_dtypes
    hidden16 = _to_bf16(inputs["hidden"])                    # [B, L, 600]
    hid_f = np.asarray(inputs["hidden"], np.float32)
    # ht8 [128, 6, B, L]: feature-major fp8 hidden chunks + pos chunk 5
    ht8 = np.zeros((128, NCH, B, L), ml_dtypes.float8_e4m3)
    hfT = hid_f.transpose(2, 0, 1)                           # [600, B, L]
    for c in range(HCH):
        cw = min(128, H2 - c * 128)
        ht8[0:cw, c] = hfT[c * 128:c * 128 + cw].astype(ml_dtypes.float8_e4m3)
    pos = np.concatenate(
        [np.asarray(inputs["pos1_emb"], np.float32),
         np.asarray(inputs["pos2_emb"], np.float32)], axis=2
    )                                                        # [B, L, 100]
    ht8[0:POSF, 5] = pos.transpose(2, 0, 1).astype(ml_dtypes.float8_e4m3)

    e1 = np.asarray(inputs["entity1_idx"]).astype(np.int64)
    e2 = np.asarray(inputs["entity2_idx"]).astype(np.int64)
    weights = _prep_weights(inputs)

    bix = np.arange(B)
    ent1 = hidden16[bix, e1]                                 # [B, H2] bf16
    ent2 = hidden16[bix, e2]
    in_maps = []
    for c in range(NCORES):
        sl = slice(c * BC, (c + 1) * BC)
        # hid_l: l-major [L, BC, H2]
        hid_l = np.ascontiguousarray(hidden16[sl].transpose(1, 0, 2))
        # ht8r: round-major [NR, 128, NCH * ROUND * L]
        ht8r = np.ascontiguousarray(
            ht8[:, :, sl, :].reshape(128, NCH, NR, ROUND * L)
            .transpose(2, 0, 1, 3)).reshape(NR, 128, NCH * ROUND * L)
        in_maps.append({
            "hid_l": hid_l,
            "ht8r": ht8r,
            "ent1": np.ascontiguousarray(ent1[sl]),
            "ent2": np.ascontiguousarray(ent2[sl]),
            **weights,
        })
    return in_maps


def unshard_z(zt, es):
    # zt: [NR//2, 128, ECH, 2, ROUND] with
    #   z[(2*pair + s)*ROUND + q, c*128 + p] = zt[pair, p, c, s, q]
    # except chunk 4 holds features 472:600 (overlaps chunk 3)
    z = np.transpose(np.asarray(zt, dtype=np.float32), (0, 3, 4, 2, 1))
    z = z.reshape(BC, ECH * 128)
    z = np.concatenate([z[:, 0:512], z[:, 512 + 40:640]], axis=1)
    # es: [ROUND+16, NR]; batch r*ROUND+q -> es[q, r], except the last
    # round's second half which lands in alias rows 32:48 when the last
    # round is finished in halves
    es = np.asarray(es, dtype=np.float32).copy()
    if LAST_HALVES:
        es[16:ROUND, NR - 1] = es[ROUND:ROUND + 16, NR - 1]
    den = es[:ROUND].T.reshape(BC, 1)
    return z / den


def kernel(**inputs):
    from concourse.bass_utils import run_bass_kernel_spmd

    nc = _get_nc()
    in_maps = make_in_maps(inputs)
    res = None
    for attempt in range(3):
        try:
            res = run_bass_kernel_spmd(
                nc, in_maps, core_ids=list(range(NCORES)))
            break
        except Exception:
            # the axon transport occasionally drops a run; retry
            if attempt == 2:
                raise
    _CACHE["last_res"] = res
    outs = [unshard_z(r["z"], r["esum"]) for r in res.results]
    return np.concatenate(outs, axis=0).astype(np.float32)
